# revision 6
# baseline (speedup 1.0000x reference)
"""Trainium2 Bass kernel v2 for nn_Attention_65609920414302.

Same math as the baseline (see kernel.py docstring) with a restructured
schedule aimed at keeping the PE continuously busy (pstate ramp) and cutting
DVE/ACT/gpsimd totals:

- RoPE partner swap via DVE stream_shuffle (channel pairs interleaved on
  partitions, partner = p^1) instead of PE swap-matmuls + PSW tile.
- Per-key score scale m = sqrt(C2)/sqrt(DH*key_self) folded into krope
  (one broadcast + one multiply) so the Square activation runs with a
  scalar scale/bias and batches across key-block pairs: 5 ACT ops per
  (wave, head) instead of 8, packed into [128,1024] PSUM tiles.
- wbuf column order groups key blocks (0),(1,7),(2,6),(3,5),(4) so paired
  blocks share one PSUM tile and one Square op.
- +delta applied as ONE tensor_scalar per (wave, head) over [128,4608].
- sink offset tb rides the PV accumulation as a 1-partition matmul
  (lhsT=tb, rhs=ones) instead of a DVE add; 1/total read straight from
  PSUM with reciprocal_approx_fast.
- W_O merged into one end pass (no y_acc staging), fp16 weights/ctx,
  fp16 output casts on gpsimd, fp16 YT writeback.
- Scores of wave j interleave with PV of wave j-1 and with V/Q2/Q3
  projection fillers in PE issue order.
"""

import math
import os
import numpy as np

D_MODEL = 1024
N_HEAD = 16
N_BR = 4
DH = 64
T = 1024
S = math.pi / math.sqrt(3.0)
# minimax quadratic fit of g(x) = silu(S*softplus(x)) over x in [-0.70, 0.70]
C2 = 0.30301553
C1 = 0.90500395
C0 = 0.97984591
SQ_BETA = C1 / (2.0 * math.sqrt(C2))
SQ_DELTA = C0 - SQ_BETA * SQ_BETA
N_CORES = 8
KT = 8

# wbuf column layout: block order (0),(1,7),(2,6),(3,5),(4)
# tiles: list of (blocks, widths-within-tile)
SC_TILES = [
    [(0, 0)],            # block, col-offset inside tile
    [(1, 0), (7, 896)],
    [(2, 0), (6, 768)],
    [(3, 0), (5, 640)],
    [(4, 0)],
]
TILE_W = [1024, 1024, 1024, 1024, 512]
TILE_OFF = [0, 1024, 2048, 3072, 4096]
BOFF = {}
for _ti, _blks in enumerate(SC_TILES):
    for _b, _o in _blks:
        BOFF[_b] = TILE_OFF[_ti] + _o
W_COLS = 4608

XMASK = [i ^ 1 for i in range(32)]

_NC_CACHE = [None]
LAST_RESULT = [None]


def _build_nc():
    import concourse.bass as bass
    from concourse import bacc
    import concourse.mybir as mybir
    import concourse.tile as tile

    F32 = mybir.dt.float32
    F16 = mybir.dt.float16
    AF = mybir.ActivationFunctionType
    ALU = mybir.AluOpType

    nc = bacc.Bacc(None, target_bir_lowering=False, debug=False)

    XT = nc.declare_dram_parameter("XT", [D_MODEL, T], F16, isOutput=False)
    WQ = nc.declare_dram_parameter("WQ", [128, 4 * KT * 128], F16, isOutput=False)
    BQ = nc.declare_dram_parameter("BQ", [128, 4], F32, isOutput=False)
    WK = nc.declare_dram_parameter("WK", [128, KT * 128], F16, isOutput=False)
    BK = nc.declare_dram_parameter("BK", [128, 1], F32, isOutput=False)
    WV = nc.declare_dram_parameter("WV", [128, KT * 128], F16, isOutput=False)
    BV = nc.declare_dram_parameter("BV", [1, 128], F16, isOutput=False)
    WO = nc.declare_dram_parameter("WO", [128, 4 * 8 * 128], F16, isOutput=False)
    COS = nc.declare_dram_parameter("COS", [128, T], F16, isOutput=False)
    SIN = nc.declare_dram_parameter("SIN", [128, T], F16, isOutput=False)
    SEL = nc.declare_dram_parameter("SEL", [128, 2], F16, isOutput=False)
    SELT = nc.declare_dram_parameter("SELT", [2, 128], F16, isOutput=False)
    TB32 = nc.declare_dram_parameter("TB32", [1, 8], F32, isOutput=False)
    VNS = nc.declare_dram_parameter("VNS", [64, 8], F32, isOutput=False)
    ONES = nc.declare_dram_parameter("ONES", [1, T], F16, isOutput=False)
    TRI = nc.declare_dram_parameter("TRI", [128, 128], F16, isOutput=False)
    YT = nc.declare_dram_parameter("YT", [D_MODEL, T], F16, isOutput=True)

    with tile.TileContext(nc) as tc:
        pc = tc.alloc_tile_pool(name="const", bufs=1)
        pk = tc.alloc_tile_pool(name="keep", bufs=1)
        tr = tc.alloc_tile_pool(name="trans", bufs=2)
        pw = tc.alloc_tile_pool(name="wbuf", bufs=1)
        pa = tc.alloc_tile_pool(name="ps", bufs=1, space="PSUM")

        # ---- constants / weights in SBUF ----
        cos_sb = pc.tile([128, T], F16)
        sin_sb = pc.tile([128, T], F16)
        sel_sb = pc.tile([128, 2], F16)
        selt_sb = pc.tile([2, 128], F16)
        tb_sb = pc.tile([1, 8], F32)
        vns_sb = pc.tile([64, 8], F32)
        ones_r = pc.tile([1, T], F16)
        beta_sb = pc.tile([128, 1], F32)
        tri_sb = pc.tile([128, 128], F16)
        nc.vector.memset(beta_sb, SQ_BETA)
        warm = pc.tile([1, 1], F32)
        nc.vector.memset(warm, 1.0)
        nc.scalar.activation(warm, warm, AF.Sqrt)

        xt = pk.tile([128, KT, T], F16)
        wk = pk.tile([128, KT, 128], F16)
        bk = pk.tile([128, 1], F32)
        wq = pk.tile([128, 4, KT, 128], F16)
        bq = pk.tile([128, 4], F32)
        wv = pk.tile([128, KT, 128], F16)
        bv = pk.tile([1, 128], F16)
        wo = pk.tile([128, 4, 8, 128], F16)

        # DMA order: K-proj deps first, then Q/rope, V, W_O last.
        xt_src = XT.ap().rearrange("(kt p) t -> p kt t", p=128)
        wk_src = WK.ap().rearrange("(kt p) m -> p kt m", p=128)
        wq_src = WQ.ap().rearrange("(kt p) (g m) -> p kt g m", p=128, m=128)
        wv_src = WV.ap().rearrange("(kt p) v -> p kt v", p=128)
        # DMA-in: single issue stream, strict priority order. All queues
        # share bandwidth; descriptors drain roughly in issue order, so the
        # K-projection critical path (wk, xt) must be issued first.
        wq_im = WQ.ap().rearrange("p (g kt m) -> p g kt m", g=4, m=128)
        wo_im = WO.ap().rearrange("p (ct mt m) -> p ct mt m", ct=4, m=128)
        nc.sync.dma_start(
            out=wk, in_=WK.ap().rearrange("p (kt m) -> p kt m", m=128))
        nc.sync.dma_start(out=bk, in_=BK.ap())
        for kt in range(KT):
            nc.sync.dma_start(out=xt[:, kt, :], in_=xt_src[:, kt, :])
        nc.sync.dma_start(out=wq[:, 0, :, :], in_=wq_im[:, 0, :, :])
        nc.sync.dma_start(out=cos_sb, in_=COS.ap())
        nc.sync.dma_start(out=sin_sb, in_=SIN.ap())
        nc.sync.dma_start(out=sel_sb, in_=SEL.ap())
        nc.sync.dma_start(out=selt_sb, in_=SELT.ap())
        nc.sync.dma_start(out=bq, in_=BQ.ap())
        nc.sync.dma_start(out=wq[:, 1, :, :], in_=wq_im[:, 1, :, :])
        nc.sync.dma_start(
            out=wv, in_=WV.ap().rearrange("p (kt m) -> p kt m", m=128))
        nc.sync.dma_start(out=bv, in_=BV.ap())
        nc.sync.dma_start(out=ones_r, in_=ONES.ap())
        nc.sync.dma_start(out=wq[:, 2, :, :], in_=wq_im[:, 2, :, :])
        nc.sync.dma_start(out=wq[:, 3, :, :], in_=wq_im[:, 3, :, :])
        nc.sync.dma_start(out=tri_sb, in_=TRI.ap())
        nc.sync.dma_start(out=tb_sb, in_=TB32.ap())
        nc.sync.dma_start(out=vns_sb, in_=VNS.ap())
        nc.sync.dma_start(out=wo, in_=wo_im)

        krope = pk.tile([128, T], F16)
        qrope = pk.tile([128, 4, T], F16)
        m16 = pk.tile([2, T], F16)
        vstore = pk.tile([128, 8, 2, 65], F16)
        ctx = pk.tile([128, 4, T], F16)
        nc.vector.memset(vstore[:, :, :, 64:65], 1.0)

        # ---- projection emitters (PE part / vector part split) ----
        def proj_mm(w_ap):
            ps = pa.tile([128, T], F32, tag="sc", bufs=2)
            for th in range(2):
                sl = slice(512 * th, 512 * (th + 1))
                for kt in range(KT):
                    nc.tensor.matmul(
                        ps[:, sl], w_ap(kt), xt[:, kt, sl],
                        start=(kt == 0), stop=(kt == KT - 1),
                    )
            return ps

        def rope_vec(ps, b_col, out_ap, add_engine, k2_out=None):
            qsb = tr.tile([128, T], F16, tag="qsb", bufs=2)
            nc.vector.tensor_scalar_add(qsb, ps, b_col)
            if k2_out is not None:
                nc.vector.tensor_tensor(k2_out, qsb, qsb, op=ALU.mult)
            sw = tr.tile([128, T], F16, tag="sw", bufs=2)
            nc.vector.stream_shuffle(sw, qsb, XMASK)
            t1 = tr.tile([128, T], F16, tag="t1", bufs=2)
            nc.vector.tensor_tensor(t1, qsb, cos_sb, op=ALU.mult)
            t2 = tr.tile([128, T], F16, tag="t2", bufs=2)
            nc.vector.tensor_tensor(t2, sw, sin_sb, op=ALU.mult)
            add_engine.tensor_tensor(out_ap, t1, t2, op=ALU.add)

        # ---- startup: K proj, Q0/Q1 proj, key_self -> m -> krope scaled ----
        ps_k = proj_mm(lambda kt: wk[:, kt, :])
        k2 = tr.tile([128, T], F16, tag="k2", bufs=1)
        rope_vec(ps_k, bk[:, 0:1], krope, nc.vector, k2_out=k2)

        ks_ps = pa.tile([2, T], F32, tag="pv", bufs=2)
        for th in range(2):
            sl = slice(512 * th, 512 * (th + 1))
            nc.tensor.matmul(ks_ps[:, sl], sel_sb, k2[:, sl],
                             start=True, stop=True)
        ps_q0 = proj_mm(lambda kt: wq[:, 0, kt, :])
        m32 = tr.tile([2, T], F32, tag="m32", bufs=1)
        nc.vector.reciprocal_approx_fast(m32, ks_ps)
        # m = sqrt(C2/DH * 1/key_self)
        nc.scalar.activation(m16, m32, AF.Sqrt, scale=C2 / DH)

        rope_vec(ps_q0, bq[:, 0:1], qrope[:, 0, :], nc.vector)
        ps_q1 = proj_mm(lambda kt: wq[:, 1, kt, :])
        # broadcast m16 rows to 64-partition halves with one PE matmul
        mb_ps = pa.tile([128, T], F32, tag="sc", bufs=2, name="mb_ps")
        for th in range(2):
            sl = slice(512 * th, 512 * (th + 1))
            nc.tensor.matmul(mb_ps[:, sl], selt_sb, m16[:, sl],
                             start=True, stop=True)
        # krope scaled in place
        nc.vector.tensor_tensor(krope, krope, mb_ps, op=ALU.mult)
        rope_vec(ps_q1, bq[:, 1:2], qrope[:, 1, :], nc.vector)

        # ---- wave machinery ----
        wbuf_of = {}

        def emit_scores_tile(j, u, ti):
            """PE matmuls for one PSUM score tile + its Square."""
            wbuf = wbuf_of[j]
            r0 = 64 * u
            st = pa.tile([128, T], F32, tag="sc", bufs=2)
            w = TILE_W[ti]
            for (b, off) in SC_TILES[ti]:
                t0 = 128 * b
                L = T - t0
                c0 = 0
                while c0 < L:
                    # stay within 512-col PSUM bank regions of the tile
                    c1 = min(c0 + 512 - ((off + c0) % 512), L)
                    nc.tensor.matmul(
                        st[:, off + c0:off + c1],
                        krope[r0:r0 + 64, t0:t0 + 128],
                        qrope[r0:r0 + 64, j, t0 + c0:t0 + c1],
                        start=True, stop=True,
                    )
                    c0 = c1
            nc.scalar.activation(
                wbuf[:, u, TILE_OFF[ti]:TILE_OFF[ti] + w], st[:, 0:w],
                AF.Square, scale=1.0, bias=beta_sb[:, 0:1],
            )

        def emit_post_scores(j, u):
            """delta add (DVE) + causal masks split across DVE/gpsimd."""
            wbuf = wbuf_of[j]
            nc.vector.tensor_scalar_add(wbuf[:, u, :], wbuf[:, u, :], SQ_DELTA)
            for b in range(8):
                o = BOFF[b]
                nc.vector.tensor_tensor(
                    wbuf[:, u, o:o + 128], wbuf[:, u, o:o + 128], tri_sb,
                    op=ALU.mult,
                )

        pv_ps = {}

        def emit_pv_units(j, u):
            """PE matmul units for PV of (wave j, head u): [regionA, regionB]."""
            wbuf = wbuf_of[j]
            h = 2 * j + u
            ps_pv = pa.tile([65, T], F32, tag="pv", bufs=2)
            pv_ps[(j, u)] = ps_pv

            def regionA():
                first = True
                for b in (0, 1, 2):
                    t0 = 128 * b
                    nc.tensor.matmul(
                        ps_pv[:, t0:512],
                        vstore[:, b, u, :],
                        wbuf[:, u, BOFF[b]:BOFF[b] + (512 - t0)],
                        start=first, stop=False,
                    )
                    first = False
                nc.tensor.matmul(
                    ps_pv[:, 384:512],
                    vstore[:, 3, u, :],
                    wbuf[:, u, BOFF[3]:BOFF[3] + 128],
                    start=False, stop=True,
                )

            def regionB():
                first = True
                for b in range(8):
                    t0 = 128 * b
                    lo = max(512, t0)
                    nc.tensor.matmul(
                        ps_pv[:, lo:T],
                        vstore[:, b, u, :],
                        wbuf[:, u, BOFF[b] + (lo - t0):BOFF[b] + (T - t0)],
                        start=first, stop=(b == 7),
                    )
                    first = False

            return [regionA, regionB]

        def emit_pv_post(j, u):
            """recip (DVE) -> gb (gpsimd) -> ctx stt (DVE)."""
            h = 2 * j + u
            r0 = 64 * u
            ps_pv = pv_ps[(j, u)]
            tt = tr.tile([1, T], F32, tag="tt", bufs=2)
            nc.scalar.activation(tt, ps_pv[64:65, :], AF.Identity,
                                 bias=tb_sb[0:1, h:h + 1])
            tp = tr.tile([1, T], F32, tag="tp", bufs=2)
            nc.vector.reciprocal_approx_fast(tp, tt)
            gb = tr.tile([64, T], F32, tag="gb", bufs=2)
            nc.gpsimd.partition_broadcast(gb, tp, channels=64)
            nc.vector.scalar_tensor_tensor(
                out=ctx[r0:r0 + 64, j, :], in0=ps_pv[0:64, :],
                scalar=vns_sb[:, h:h + 1], in1=gb,
                op0=ALU.add, op1=ALU.mult,
            )

        # ---- V projection units (PE) + copies (DVE) ----
        def v_unit(tt_i):
            def emit():
                psv = pa.tile([128, T], F32, tag="sc", bufs=2)
                for kt in range(KT):
                    nc.tensor.matmul(
                        psv[:, 0:128], xt[:, kt, 128 * tt_i:128 * (tt_i + 1)],
                        wv[:, kt, :], start=(kt == 0), stop=False,
                    )
                nc.tensor.matmul(
                    psv[:, 0:128], ones_r[0:1, 0:128], bv,
                    start=False, stop=True,
                )
                nc.vector.tensor_copy(
                    vstore[:, tt_i, :, 0:64],
                    psv[:, 0:128].rearrange("p (h d) -> p h d", d=64),
                )
            return emit

        # ---- Q2/Q3 projection split into PE quarter-units + vector part ----
        def q_quarter(g, th, kts):
            def emit():
                ps = qps[g]
                sl = slice(512 * th, 512 * (th + 1))
                for kt in kts:
                    nc.tensor.matmul(
                        ps[:, sl], wq[:, g, kt, :], xt[:, kt, sl],
                        start=(kt == 0), stop=(kt == KT - 1),
                    )
            return emit

        qps = {}

        # ---------------- wave 0 ----------------
        wbuf_of[0] = pw.tile([128, 2, W_COLS], F16, tag="wbuf", bufs=3,
                             name="wbuf0")
        qps[2] = pa.tile([128, T], F32, tag="pv", bufs=2, name="qps2")
        fillers = [q_quarter(2, th, kts) for th in range(2)
                   for kts in (range(0, 4), range(4, 8))]
        fillers += [v_unit(i) for i in range(8)]
        fi = 0
        for u in range(2):
            for ti in range(5):
                if u == 1 and ti == 0 and fi < len(fillers):
                    fillers[fi]()
                    fi += 1
                emit_scores_tile(0, u, ti)
                if fi < len(fillers):
                    fillers[fi]()
                    fi += 1
                if (u == 1 or ti == 4) and fi < len(fillers):
                    fillers[fi]()
                    fi += 1
            emit_post_scores(0, u)
            if u == 0:
                rope_vec(qps[2], bq[:, 2:3], qrope[:, 2, :], nc.gpsimd)
        while fi < len(fillers):
            fillers[fi]()
            fi += 1

        # ---------------- waves 1..3 ----------------
        for j in (1, 2, 3):
            wbuf_of[j] = pw.tile([128, 2, W_COLS], F16, tag="wbuf", bufs=3,
                                 name=f"wbuf{j}")
            fillers = []
            if j == 1:
                qps[3] = pa.tile([128, T], F32, tag="pv", bufs=2, name="qps3")
                fillers = [q_quarter(3, th, kts) for th in range(2)
                           for kts in (range(0, 4), range(4, 8))]
            units0 = emit_pv_units(j - 1, 0)
            units1 = emit_pv_units(j - 1, 1)
            seq = fillers + [units0[0], units0[1], ("post", 0),
                             units1[0], units1[1], ("post", 1)]
            si = 0

            def take():
                nonlocal si, seq
                if si < len(seq):
                    it = seq[si]
                    si += 1
                    if isinstance(it, tuple):
                        # emit each post right after its units so the
                        # recip/gb/stt chain enters the DVE/gpsimd queues
                        # ahead of the next delta+masks and frees the PV
                        # PSUM slot a half-wave earlier
                        emit_pv_post(j - 1, it[1])
                    else:
                        it()

            for u in range(2):
                for ti in range(5):
                    if u == 1 and ti == 0:
                        # run the PV unit BEFORE the half's first tile: it
                        # executes while ACT drains the u0 squares that this
                        # tile's PSUM slot is waiting on
                        take()
                    emit_scores_tile(j, u, ti)
                    if j == 1:
                        take()
                    elif (u == 0 and ti in (1, 3, 4)) or                          (u == 1 and ti in (2, 4)):
                        take()
                emit_post_scores(j, u)
                if j == 1 and u == 0:
                    rope_vec(qps[3], bq[:, 3:4], qrope[:, 3, :], nc.gpsimd)
            while si < len(seq):
                take()

        # ---------------- PV of wave 3 + W_O ----------------
        for unit in emit_pv_units(3, 0):
            unit()
        emit_pv_post(3, 0)
        # two W_O chains (branches 0-2 parts) started on the free sc slots:
        # they execute while wave-3 u1's mask/PV/normalize chain drains
        pre_ps = []
        for bi in range(2):
            mt, th = bi // 2, bi % 2
            sl = slice(512 * th, 512 * (th + 1))
            ps_o = pa.tile([128, 512], F32, tag="sc", bufs=2, name="ps_o")
            for ci in range(3):
                nc.tensor.matmul(
                    ps_o, wo[:, ci, mt, :], ctx[:, ci, sl],
                    start=(ci == 0), stop=False,
                )
            pre_ps.append((mt, sl, ps_o))
        for unit in emit_pv_units(3, 1):
            unit()
        # two more chains on the pv slots freed by stt(3,0) mid-drain
        for bi in range(2, 4):
            mt, th = bi // 2, bi % 2
            sl = slice(512 * th, 512 * (th + 1))
            ps_o = pa.tile([128, 512], F32, tag="pv", bufs=2, name="ps_o")
            for ci in range(3):
                nc.tensor.matmul(
                    ps_o, wo[:, ci, mt, :], ctx[:, ci, sl],
                    start=(ci == 0), stop=False,
                )
            pre_ps.append((mt, sl, ps_o))
        emit_pv_post(3, 1)

        def wo_finish(mt, sl, ps_o):
            nc.tensor.matmul(
                ps_o, wo[:, 3, mt, :], ctx[:, 3, sl],
                start=False, stop=True,
            )
            ysb = tr.tile([128, 512], F16, tag="ysb", bufs=3)
            nc.vector.tensor_copy(ysb, ps_o)
            nc.sync.dma_start(
                out=YT.ap()[128 * mt:128 * (mt + 1), sl], in_=ysb
            )

        for (mt, sl, ps_o) in pre_ps:
            wo_finish(mt, sl, ps_o)
        for bi in range(4, 16):
            mt, th = bi // 2, bi % 2
            sl = slice(512 * th, 512 * (th + 1))
            tag = "sc" if bi % 2 == 1 else "pv"
            ps_o = pa.tile([128, 512], F32, tag=tag, bufs=2, name="ps_o")
            for ci in range(4):
                nc.tensor.matmul(
                    ps_o, wo[:, ci, mt, :], ctx[:, ci, sl],
                    start=(ci == 0), stop=(ci == 3),
                )
            ysb = tr.tile([128, 512], F16, tag="ysb", bufs=3)
            nc.vector.tensor_copy(ysb, ps_o)
            nc.sync.dma_start(
                out=YT.ap()[128 * mt:128 * (mt + 1), sl], in_=ysb
            )

        pa.release()
        pw.release()
        tr.release()
        pk.release()
        pc.release()

    # pin Sqrt+Square to one table set so the picker never splits them
    import concourse.bacc as _bacc_mod
    from concourse.hw_specs import get_activation_tables as _gat
    AFT = mybir.ActivationFunctionType

    def _gat_patched(arch):
        t = {k: set(v) for k, v in _gat(arch).items()}
        if "sqrt_and_others" in t:
            for k in t:
                if k != "sqrt_and_others":
                    t[k].discard(AFT.Sqrt)
                    t[k].discard(AFT.Square)
        return t

    _bacc_mod.get_activation_tables = _gat_patched
    try:
        nc.finalize()
    finally:
        _bacc_mod.get_activation_tables = _gat
    return nc


def _host_inputs(inputs):
    X = np.asarray(inputs["X"], dtype=np.float32)
    W_Q = np.asarray(inputs["W_Q"], dtype=np.float32)
    b_Q = np.asarray(inputs["b_Q"], dtype=np.float32)
    W_K = np.asarray(inputs["W_K"], dtype=np.float32)
    b_K = np.asarray(inputs["b_K"], dtype=np.float32)
    W_V = np.asarray(inputs["W_V"], dtype=np.float32)
    b_V = np.asarray(inputs["b_V"], dtype=np.float32)
    sink = np.asarray(inputs["sink_scalars"], dtype=np.float32)
    v_nulls = np.asarray(inputs["v_nulls"], dtype=np.float32)
    W_O = np.asarray(inputs["W_O"], dtype=np.float32)

    XT = np.ascontiguousarray(X[0].T)

    # RoPE tables for interleaved channel-pair layout: row 2i -> (cos_i, -sin_i),
    # row 2i+1 -> (cos_i, +sin_i); partner = row ^ 1.
    invf = (1.0 / (10000.0 ** (np.arange(0, DH, 2, dtype=np.float32) / DH))
            ).astype(np.float32)
    freqs = np.arange(T, dtype=np.float32)[:, None] * invf[None, :]  # [T, 32]
    cos32 = np.cos(freqs).T  # [32, T]
    sin32 = np.sin(freqs).T
    cos64 = np.repeat(cos32, 2, axis=0)                      # [64, T]
    sin64 = np.stack([-sin32, sin32], axis=1).reshape(64, T)  # [64, T]
    cos128 = np.tile(cos64, (2, 1)).astype(np.float16)
    sin128 = np.tile(sin64, (2, 1)).astype(np.float16)

    sel = np.zeros((128, 2), dtype=np.float16)
    sel[0:64, 0] = 1.0
    sel[64:128, 1] = 1.0
    selt = np.zeros((2, 128), dtype=np.float16)
    selt[0, 0:64] = 1.0
    selt[1, 64:128] = 1.0

    in_maps = []
    for c in range(N_CORES):
        kcols = np.arange(128 * c, 128 * c + 128)
        wq_blocks, bq_cols = [], []
        for n in range(N_BR):
            qb = np.arange(1024 * n + 128 * c, 1024 * n + 128 * c + 128)
            wq_blocks.append(W_Q[:, qb])
            bq_cols.append(b_Q[qb])
        wq_full = np.concatenate(wq_blocks, axis=1)          # [1024, 512]
        bq_full = np.stack(bq_cols, axis=1)                  # [128, 4]
        # SBUF-image: [p, g, kt, m]
        wq_im = wq_full.reshape(KT, 128, 4, 128).transpose(1, 2, 0, 3)
        wq_im = wq_im.reshape(128, 4 * KT * 128)
        wo_full = np.concatenate(
            [0.25 * W_O[n, 128 * c:128 * c + 128, :] for n in range(N_BR)],
            axis=0,
        )                                                    # [512, 1024]
        # SBUF-image: [p, ct, mt, m]
        wo_im = wo_full.reshape(4, 128, 8, 128).transpose(1, 0, 2, 3)
        wo_im = wo_im.reshape(128, 4 * 8 * 128)
        heads = np.array([16 * n + 2 * c + u for n in range(N_BR)
                          for u in range(2)])
        sinks = sink[heads]
        tb = (S * (sinks + 1e-6)).astype(np.float32)[None, :]
        vns = np.zeros((64, 8), dtype=np.float32)
        for n in range(N_BR):
            for u in range(2):
                hl = 2 * n + u
                vns[:, hl] = S * sinks[hl] * v_nulls[n].reshape(N_HEAD, DH)[
                    2 * c + u
                ]
        in_maps.append(
            {
                "XT": XT.astype(np.float16),
                "WQ": np.ascontiguousarray(wq_im).astype(np.float16),
                "BQ": np.ascontiguousarray(bq_full).astype(np.float32),
                "WK": np.ascontiguousarray(
                    W_K[:, kcols].reshape(KT, 128, 128).transpose(1, 0, 2)
                    .reshape(128, KT * 128)).astype(np.float16),
                "BK": np.ascontiguousarray(b_K[kcols][:, None]).astype(
                    np.float32
                ),
                "WV": np.ascontiguousarray(
                    W_V[:, kcols].reshape(KT, 128, 128).transpose(1, 0, 2)
                    .reshape(128, KT * 128)).astype(np.float16),
                "BV": np.ascontiguousarray(b_V[kcols])[None, :].astype(
                    np.float16
                ),
                "WO": np.ascontiguousarray(wo_im).astype(np.float16),
                "COS": cos128,
                "SIN": sin128,
                "SEL": sel,
                "SELT": selt,
                "TB32": tb,
                "VNS": vns,
                "ONES": np.ones((1, T), dtype=np.float16),
                "TRI": np.tril(np.ones((128, 128))).astype(np.float16).T.copy(),
            }
        )
    return in_maps


def kernel(**inputs) -> np.ndarray:
    from concourse.bass_utils import run_bass_kernel_spmd

    in_maps = _host_inputs(inputs)
    if _NC_CACHE[0] is None:
        _NC_CACHE[0] = _build_nc()
    nc = _NC_CACHE[0]
    trace = bool(os.environ.get("KBENCH_TRACE"))
    res = run_bass_kernel_spmd(
        nc, in_maps, core_ids=list(range(N_CORES)), trace=trace
    )
    LAST_RESULT[0] = res
    if trace and res.exec_time_ns is not None:
        print(f"HW exec time: {res.exec_time_ns} ns")

    W_O_bias = np.asarray(inputs["W_O_bias"], dtype=np.float32)
    y = np.zeros((T, D_MODEL), dtype=np.float32)
    for r in res.results:
        y += np.asarray(r["YT"], dtype=np.float32).T
    y += W_O_bias.mean(axis=0)[None, :]
    return y[None, :, :]


# revision 7
# speedup vs baseline: 1.0436x; 1.0436x over previous
"""Trainium2 Bass kernel v2 for nn_Attention_65609920414302.

Same math as the baseline (see kernel.py docstring) with a restructured
schedule aimed at keeping the PE continuously busy (pstate ramp) and cutting
DVE/ACT/gpsimd totals:

- RoPE partner swap via DVE stream_shuffle (channel pairs interleaved on
  partitions, partner = p^1) instead of PE swap-matmuls + PSW tile.
- Per-key score scale m = sqrt(C2)/sqrt(DH*key_self) folded into krope
  (one broadcast + one multiply) so the Square activation runs with a
  scalar scale/bias and batches across key-block pairs: 5 ACT ops per
  (wave, head) instead of 8, packed into [128,1024] PSUM tiles.
- wbuf column order groups key blocks (0),(1,7),(2,6),(3,5),(4) so paired
  blocks share one PSUM tile and one Square op.
- +delta applied as ONE tensor_scalar per (wave, head) over [128,4608].
- sink offset tb rides the PV accumulation as a 1-partition matmul
  (lhsT=tb, rhs=ones) instead of a DVE add; 1/total read straight from
  PSUM with reciprocal_approx_fast.
- W_O merged into one end pass (no y_acc staging), fp16 weights/ctx,
  fp16 output casts on gpsimd, fp16 YT writeback.
- Scores of wave j interleave with PV of wave j-1 and with V/Q2/Q3
  projection fillers in PE issue order.
"""

import math
import os
import numpy as np

D_MODEL = 1024
N_HEAD = 16
N_BR = 4
DH = 64
T = 1024
S = math.pi / math.sqrt(3.0)
# minimax quadratic fit of g(x) = silu(S*softplus(x)) over x in [-0.70, 0.70]
C2 = 0.30301553
C1 = 0.90500395
C0 = 0.97984591
SQ_BETA = C1 / (2.0 * math.sqrt(C2))
SQ_DELTA = C0 - SQ_BETA * SQ_BETA
N_CORES = 8
KT = 8

# wbuf column layout: block order (0),(1,7),(2,6),(3,5),(4)
# tiles: list of (blocks, widths-within-tile)
SC_TILES = [
    [(0, 0)],            # block, col-offset inside tile
    [(1, 0), (7, 896)],
    [(2, 0), (6, 768)],
    [(3, 0), (5, 640)],
    [(4, 0)],
]
TILE_W = [1024, 1024, 1024, 1024, 512]
TILE_OFF = [0, 1024, 2048, 3072, 4096]
BOFF = {}
for _ti, _blks in enumerate(SC_TILES):
    for _b, _o in _blks:
        BOFF[_b] = TILE_OFF[_ti] + _o
W_COLS = 4608

XMASK = [i ^ 1 for i in range(32)]

_NC_CACHE = [None]
LAST_RESULT = [None]


def _build_nc():
    import concourse.bass as bass
    from concourse import bacc
    import concourse.mybir as mybir
    import concourse.tile as tile

    F32 = mybir.dt.float32
    F16 = mybir.dt.float16
    AF = mybir.ActivationFunctionType
    ALU = mybir.AluOpType

    nc = bacc.Bacc(None, target_bir_lowering=False, debug=False)

    XT = nc.declare_dram_parameter("XT", [D_MODEL, T], F16, isOutput=False)
    WQ = nc.declare_dram_parameter("WQ", [128, 4 * KT * 128], F16, isOutput=False)
    BQ = nc.declare_dram_parameter("BQ", [128, 4], F32, isOutput=False)
    WK = nc.declare_dram_parameter("WK", [128, KT * 128], F16, isOutput=False)
    BK = nc.declare_dram_parameter("BK", [128, 1], F32, isOutput=False)
    WV = nc.declare_dram_parameter("WV", [128, KT * 128], F16, isOutput=False)
    BV = nc.declare_dram_parameter("BV", [1, 128], F16, isOutput=False)
    WO = nc.declare_dram_parameter("WO", [128, 4 * 8 * 128], F16, isOutput=False)
    COS = nc.declare_dram_parameter("COS", [128, T], F16, isOutput=False)
    SIN = nc.declare_dram_parameter("SIN", [128, T], F16, isOutput=False)
    SEL = nc.declare_dram_parameter("SEL", [128, 2], F16, isOutput=False)
    SELT = nc.declare_dram_parameter("SELT", [2, 128], F16, isOutput=False)
    TB32 = nc.declare_dram_parameter("TB32", [1, 8], F32, isOutput=False)
    VNS = nc.declare_dram_parameter("VNS", [64, 8], F32, isOutput=False)
    ONES = nc.declare_dram_parameter("ONES", [1, T], F16, isOutput=False)
    TRI = nc.declare_dram_parameter("TRI", [128, 128], F16, isOutput=False)
    YT = nc.declare_dram_parameter("YT", [D_MODEL, T], F16, isOutput=True)

    with tile.TileContext(nc) as tc:
        pc = tc.alloc_tile_pool(name="const", bufs=1)
        pk = tc.alloc_tile_pool(name="keep", bufs=1)
        tr = tc.alloc_tile_pool(name="trans", bufs=2)
        pw = tc.alloc_tile_pool(name="wbuf", bufs=1)
        pa = tc.alloc_tile_pool(name="ps", bufs=1, space="PSUM")

        # ---- constants / weights in SBUF ----
        cos_sb = pc.tile([128, T], F16)
        sin_sb = pc.tile([128, T], F16)
        sel_sb = pc.tile([128, 2], F16)
        selt_sb = pc.tile([2, 128], F16)
        tb_sb = pc.tile([1, 8], F32)
        vns_sb = pc.tile([64, 8], F32)
        ones_r = pc.tile([1, T], F16)
        beta_sb = pc.tile([128, 1], F32)
        tri_sb = pc.tile([128, 128], F16)
        nc.vector.memset(beta_sb, SQ_BETA)
        warm = pc.tile([1, 1], F32)
        nc.vector.memset(warm, 1.0)
        nc.scalar.activation(warm, warm, AF.Sqrt)
        # PE warm-up: ~20 dummy matmuls on memset constants (no DMA deps)
        # keep the PE busy from t~2us so the frequency governor is already
        # ramped when the DMA-gated projections start at t~11us
        warm64 = pc.tile([1, 512], F16)
        warmw = pc.tile([1, 1], F16)
        nc.vector.memset(warm64, 1.0)
        nc.vector.memset(warmw, 1.0)
        warm_ps = pa.tile([128, T], F32, tag="sc", bufs=2, name="warm_ps")
        for _ in range(14):
            nc.tensor.matmul(warm_ps[0:1, 0:512], warmw, warm64,
                             start=True, stop=True)
        warm_rd = pc.tile([1, 64], F32)
        nc.vector.tensor_copy(warm_rd, warm_ps[0:1, 0:64])

        xt = pk.tile([128, KT, T], F16)
        wk = pk.tile([128, KT, 128], F16)
        bk = pk.tile([128, 1], F32)
        wq = pk.tile([128, 4, KT, 128], F16)
        bq = pk.tile([128, 4], F32)
        wv = pk.tile([128, KT, 128], F16)
        bv = pk.tile([1, 128], F16)
        wo = pk.tile([128, 4, 8, 128], F16)

        # DMA order: K-proj deps first, then Q/rope, V, W_O last.
        xt_src = XT.ap().rearrange("(kt p) t -> p kt t", p=128)
        wk_src = WK.ap().rearrange("(kt p) m -> p kt m", p=128)
        wq_src = WQ.ap().rearrange("(kt p) (g m) -> p kt g m", p=128, m=128)
        wv_src = WV.ap().rearrange("(kt p) v -> p kt v", p=128)
        # DMA-in: single issue stream, strict priority order. All queues
        # share bandwidth; descriptors drain roughly in issue order, so the
        # K-projection critical path (wk, xt) must be issued first.
        wq_im = WQ.ap().rearrange("p (g kt m) -> p g kt m", g=4, m=128)
        wo_im = WO.ap().rearrange("p (ct mt m) -> p ct mt m", ct=4, m=128)
        nc.sync.dma_start(
            out=wk, in_=WK.ap().rearrange("p (kt m) -> p kt m", m=128))
        nc.sync.dma_start(out=bk, in_=BK.ap())
        for kt in range(KT):
            nc.sync.dma_start(out=xt[:, kt, :], in_=xt_src[:, kt, :])
        nc.sync.dma_start(out=wq[:, 0, :, :], in_=wq_im[:, 0, :, :])
        nc.sync.dma_start(out=cos_sb, in_=COS.ap())
        nc.sync.dma_start(out=sin_sb, in_=SIN.ap())
        nc.sync.dma_start(out=sel_sb, in_=SEL.ap())
        nc.sync.dma_start(out=selt_sb, in_=SELT.ap())
        nc.sync.dma_start(out=bq, in_=BQ.ap())
        nc.sync.dma_start(out=wq[:, 1, :, :], in_=wq_im[:, 1, :, :])
        nc.sync.dma_start(
            out=wv, in_=WV.ap().rearrange("p (kt m) -> p kt m", m=128))
        nc.sync.dma_start(out=bv, in_=BV.ap())
        nc.sync.dma_start(out=ones_r, in_=ONES.ap())
        nc.sync.dma_start(out=wq[:, 2, :, :], in_=wq_im[:, 2, :, :])
        nc.sync.dma_start(out=wq[:, 3, :, :], in_=wq_im[:, 3, :, :])
        nc.sync.dma_start(out=tri_sb, in_=TRI.ap())
        nc.sync.dma_start(out=tb_sb, in_=TB32.ap())
        nc.sync.dma_start(out=vns_sb, in_=VNS.ap())
        nc.sync.dma_start(out=wo, in_=wo_im)

        krope = pk.tile([128, T], F16)
        qrope = pk.tile([128, 4, T], F16)
        m16 = pk.tile([2, T], F16)
        vstore = pk.tile([128, 8, 2, 65], F16)
        ctx = pk.tile([128, 4, T], F16)
        nc.vector.memset(vstore[:, :, :, 64:65], 1.0)

        # ---- projection emitters (PE part / vector part split) ----
        def proj_mm(w_ap):
            ps = pa.tile([128, T], F32, tag="sc", bufs=2)
            for th in range(2):
                sl = slice(512 * th, 512 * (th + 1))
                for kt in range(KT):
                    nc.tensor.matmul(
                        ps[:, sl], w_ap(kt), xt[:, kt, sl],
                        start=(kt == 0), stop=(kt == KT - 1),
                    )
            return ps

        def rope_vec(ps, b_col, out_ap, add_engine, k2_out=None):
            qsb = tr.tile([128, T], F16, tag="qsb", bufs=2)
            nc.vector.tensor_scalar_add(qsb, ps, b_col)
            if k2_out is not None:
                nc.vector.tensor_tensor(k2_out, qsb, qsb, op=ALU.mult)
            sw = tr.tile([128, T], F16, tag="sw", bufs=2)
            nc.vector.stream_shuffle(sw, qsb, XMASK)
            t1 = tr.tile([128, T], F16, tag="t1", bufs=2)
            nc.vector.tensor_tensor(t1, qsb, cos_sb, op=ALU.mult)
            t2 = tr.tile([128, T], F16, tag="t2", bufs=2)
            nc.vector.tensor_tensor(t2, sw, sin_sb, op=ALU.mult)
            add_engine.tensor_tensor(out_ap, t1, t2, op=ALU.add)

        # ---- startup: K proj, Q0/Q1 proj, key_self -> m -> krope scaled ----
        ps_k = proj_mm(lambda kt: wk[:, kt, :])
        k2 = tr.tile([128, T], F16, tag="k2", bufs=1)
        rope_vec(ps_k, bk[:, 0:1], krope, nc.vector, k2_out=k2)

        ks_ps = pa.tile([2, T], F32, tag="pv", bufs=2)
        for th in range(2):
            sl = slice(512 * th, 512 * (th + 1))
            nc.tensor.matmul(ks_ps[:, sl], sel_sb, k2[:, sl],
                             start=True, stop=True)
        ps_q0 = proj_mm(lambda kt: wq[:, 0, kt, :])
        m32 = tr.tile([2, T], F32, tag="m32", bufs=1)
        nc.vector.reciprocal_approx_fast(m32, ks_ps)
        # m = sqrt(C2/DH * 1/key_self)
        nc.scalar.activation(m16, m32, AF.Sqrt, scale=C2 / DH)

        rope_vec(ps_q0, bq[:, 0:1], qrope[:, 0, :], nc.vector)
        ps_q1 = proj_mm(lambda kt: wq[:, 1, kt, :])
        # broadcast m16 rows to 64-partition halves with one PE matmul
        mb_ps = pa.tile([128, T], F32, tag="sc", bufs=2, name="mb_ps")
        for th in range(2):
            sl = slice(512 * th, 512 * (th + 1))
            nc.tensor.matmul(mb_ps[:, sl], selt_sb, m16[:, sl],
                             start=True, stop=True)
        # krope scaled in place
        nc.vector.tensor_tensor(krope, krope, mb_ps, op=ALU.mult)
        rope_vec(ps_q1, bq[:, 1:2], qrope[:, 1, :], nc.vector)

        # ---- wave machinery ----
        wbuf_of = {}

        def emit_scores_tile(j, u, ti):
            """PE matmuls for one PSUM score tile + its Square."""
            wbuf = wbuf_of[j]
            r0 = 64 * u
            st = pa.tile([128, T], F32, tag="sc", bufs=2)
            w = TILE_W[ti]
            for (b, off) in SC_TILES[ti]:
                t0 = 128 * b
                L = T - t0
                c0 = 0
                while c0 < L:
                    # stay within 512-col PSUM bank regions of the tile
                    c1 = min(c0 + 512 - ((off + c0) % 512), L)
                    nc.tensor.matmul(
                        st[:, off + c0:off + c1],
                        krope[r0:r0 + 64, t0:t0 + 128],
                        qrope[r0:r0 + 64, j, t0 + c0:t0 + c1],
                        start=True, stop=True,
                    )
                    c0 = c1
            nc.scalar.activation(
                wbuf[:, u, TILE_OFF[ti]:TILE_OFF[ti] + w], st[:, 0:w],
                AF.Square, scale=1.0, bias=beta_sb[:, 0:1],
            )

        def emit_post_scores(j, u):
            """delta add (DVE) + causal masks split across DVE/gpsimd."""
            wbuf = wbuf_of[j]
            nc.vector.tensor_scalar_add(wbuf[:, u, :], wbuf[:, u, :], SQ_DELTA)
            for b in range(8):
                o = BOFF[b]
                nc.vector.tensor_tensor(
                    wbuf[:, u, o:o + 128], wbuf[:, u, o:o + 128], tri_sb,
                    op=ALU.mult,
                )

        pv_ps = {}

        def emit_pv_units(j, u):
            """PE matmul units for PV of (wave j, head u): [regionA, regionB]."""
            wbuf = wbuf_of[j]
            h = 2 * j + u
            ps_pv = pa.tile([65, T], F32, tag="pv", bufs=2)
            pv_ps[(j, u)] = ps_pv

            def regionA():
                first = True
                for b in (0, 1, 2):
                    t0 = 128 * b
                    nc.tensor.matmul(
                        ps_pv[:, t0:512],
                        vstore[:, b, u, :],
                        wbuf[:, u, BOFF[b]:BOFF[b] + (512 - t0)],
                        start=first, stop=False,
                    )
                    first = False
                nc.tensor.matmul(
                    ps_pv[:, 384:512],
                    vstore[:, 3, u, :],
                    wbuf[:, u, BOFF[3]:BOFF[3] + 128],
                    start=False, stop=True,
                )

            def regionB():
                first = True
                for b in range(8):
                    t0 = 128 * b
                    lo = max(512, t0)
                    nc.tensor.matmul(
                        ps_pv[:, lo:T],
                        vstore[:, b, u, :],
                        wbuf[:, u, BOFF[b] + (lo - t0):BOFF[b] + (T - t0)],
                        start=first, stop=(b == 7),
                    )
                    first = False

            return [regionA, regionB]

        def emit_pv_post(j, u):
            """recip (DVE) -> gb (gpsimd) -> ctx stt (DVE)."""
            h = 2 * j + u
            r0 = 64 * u
            ps_pv = pv_ps[(j, u)]
            tt = tr.tile([1, T], F32, tag="tt", bufs=2)
            nc.scalar.activation(tt, ps_pv[64:65, :], AF.Identity,
                                 bias=tb_sb[0:1, h:h + 1])
            tp = tr.tile([1, T], F32, tag="tp", bufs=2)
            nc.vector.reciprocal_approx_fast(tp, tt)
            gb = tr.tile([64, T], F32, tag="gb", bufs=2)
            nc.gpsimd.partition_broadcast(gb, tp, channels=64)
            nc.vector.scalar_tensor_tensor(
                out=ctx[r0:r0 + 64, j, :], in0=ps_pv[0:64, :],
                scalar=vns_sb[:, h:h + 1], in1=gb,
                op0=ALU.add, op1=ALU.mult,
            )

        # ---- V projection units (PE) + copies (DVE) ----
        def v_unit(tt_i):
            def emit():
                psv = pa.tile([128, T], F32, tag="sc", bufs=2)
                for kt in range(KT):
                    nc.tensor.matmul(
                        psv[:, 0:128], xt[:, kt, 128 * tt_i:128 * (tt_i + 1)],
                        wv[:, kt, :], start=(kt == 0), stop=False,
                    )
                nc.tensor.matmul(
                    psv[:, 0:128], ones_r[0:1, 0:128], bv,
                    start=False, stop=True,
                )
                nc.vector.tensor_copy(
                    vstore[:, tt_i, :, 0:64],
                    psv[:, 0:128].rearrange("p (h d) -> p h d", d=64),
                )
            return emit

        # ---- Q2/Q3 projection split into PE quarter-units + vector part ----
        def q_quarter(g, th, kts):
            def emit():
                ps = qps[g]
                sl = slice(512 * th, 512 * (th + 1))
                for kt in kts:
                    nc.tensor.matmul(
                        ps[:, sl], wq[:, g, kt, :], xt[:, kt, sl],
                        start=(kt == 0), stop=(kt == KT - 1),
                    )
            return emit

        qps = {}

        # ---------------- wave 0 ----------------
        wbuf_of[0] = pw.tile([128, 2, W_COLS], F16, tag="wbuf", bufs=3,
                             name="wbuf0")
        qps[2] = pa.tile([128, T], F32, tag="pv", bufs=2, name="qps2")
        fillers = [q_quarter(2, th, kts) for th in range(2)
                   for kts in (range(0, 4), range(4, 8))]
        fillers += [v_unit(i) for i in range(8)]
        fi = 0
        for u in range(2):
            for ti in range(5):
                if u == 1 and ti == 0 and fi < len(fillers):
                    fillers[fi]()
                    fi += 1
                emit_scores_tile(0, u, ti)
                if fi < len(fillers):
                    fillers[fi]()
                    fi += 1
                if (u == 1 or ti == 4) and fi < len(fillers):
                    fillers[fi]()
                    fi += 1
            emit_post_scores(0, u)
            if u == 0:
                rope_vec(qps[2], bq[:, 2:3], qrope[:, 2, :], nc.gpsimd)
        while fi < len(fillers):
            fillers[fi]()
            fi += 1

        # ---------------- waves 1..3 ----------------
        for j in (1, 2, 3):
            wbuf_of[j] = pw.tile([128, 2, W_COLS], F16, tag="wbuf", bufs=3,
                                 name=f"wbuf{j}")
            fillers = []
            if j == 1:
                qps[3] = pa.tile([128, T], F32, tag="pv", bufs=2, name="qps3")
                fillers = [q_quarter(3, th, kts) for th in range(2)
                           for kts in (range(0, 4), range(4, 8))]
            units0 = emit_pv_units(j - 1, 0)
            units1 = emit_pv_units(j - 1, 1)
            seq = fillers + [units0[0], units0[1], ("post", 0),
                             units1[0], units1[1], ("post", 1)]
            si = 0

            def take():
                nonlocal si, seq
                if si < len(seq):
                    it = seq[si]
                    si += 1
                    if isinstance(it, tuple):
                        # emit each post right after its units so the
                        # recip/gb/stt chain enters the DVE/gpsimd queues
                        # ahead of the next delta+masks and frees the PV
                        # PSUM slot a half-wave earlier
                        emit_pv_post(j - 1, it[1])
                    else:
                        it()

            for u in range(2):
                for ti in range(5):
                    if u == 1 and ti == 0:
                        # run the PV unit BEFORE the half's first tile: it
                        # executes while ACT drains the u0 squares that this
                        # tile's PSUM slot is waiting on
                        take()
                    emit_scores_tile(j, u, ti)
                    if j == 1:
                        take()
                    elif (u == 0 and ti in (1, 3, 4)) or                          (u == 1 and ti in (2, 4)):
                        take()
                emit_post_scores(j, u)
                if j == 1 and u == 0:
                    rope_vec(qps[3], bq[:, 3:4], qrope[:, 3, :], nc.gpsimd)
            while si < len(seq):
                take()

        # ---------------- PV of wave 3 + W_O ----------------
        for unit in emit_pv_units(3, 0):
            unit()
        emit_pv_post(3, 0)
        # two W_O chains (branches 0-2 parts) started on the free sc slots:
        # they execute while wave-3 u1's mask/PV/normalize chain drains
        pre_ps = []
        for bi in range(2):
            mt, th = bi // 2, bi % 2
            sl = slice(512 * th, 512 * (th + 1))
            ps_o = pa.tile([128, 512], F32, tag="sc", bufs=2, name="ps_o")
            for ci in range(3):
                nc.tensor.matmul(
                    ps_o, wo[:, ci, mt, :], ctx[:, ci, sl],
                    start=(ci == 0), stop=False,
                )
            pre_ps.append((mt, sl, ps_o))
        for unit in emit_pv_units(3, 1):
            unit()
        # two more chains on the pv slots freed by stt(3,0) mid-drain
        for bi in range(2, 4):
            mt, th = bi // 2, bi % 2
            sl = slice(512 * th, 512 * (th + 1))
            ps_o = pa.tile([128, 512], F32, tag="pv", bufs=2, name="ps_o")
            for ci in range(3):
                nc.tensor.matmul(
                    ps_o, wo[:, ci, mt, :], ctx[:, ci, sl],
                    start=(ci == 0), stop=False,
                )
            pre_ps.append((mt, sl, ps_o))
        emit_pv_post(3, 1)

        def wo_finish(mt, sl, ps_o):
            nc.tensor.matmul(
                ps_o, wo[:, 3, mt, :], ctx[:, 3, sl],
                start=False, stop=True,
            )
            ysb = tr.tile([128, 512], F16, tag="ysb", bufs=3)
            nc.vector.tensor_copy(ysb, ps_o)
            nc.sync.dma_start(
                out=YT.ap()[128 * mt:128 * (mt + 1), sl], in_=ysb
            )

        for (mt, sl, ps_o) in pre_ps:
            wo_finish(mt, sl, ps_o)
        for bi in range(4, 16):
            mt, th = bi // 2, bi % 2
            sl = slice(512 * th, 512 * (th + 1))
            tag = "sc" if bi % 2 == 1 else "pv"
            ps_o = pa.tile([128, 512], F32, tag=tag, bufs=2, name="ps_o")
            for ci in range(4):
                nc.tensor.matmul(
                    ps_o, wo[:, ci, mt, :], ctx[:, ci, sl],
                    start=(ci == 0), stop=(ci == 3),
                )
            ysb = tr.tile([128, 512], F16, tag="ysb", bufs=3)
            nc.vector.tensor_copy(ysb, ps_o)
            nc.sync.dma_start(
                out=YT.ap()[128 * mt:128 * (mt + 1), sl], in_=ysb
            )

        pa.release()
        pw.release()
        tr.release()
        pk.release()
        pc.release()

    # pin Sqrt+Square to one table set so the picker never splits them
    import concourse.bacc as _bacc_mod
    from concourse.hw_specs import get_activation_tables as _gat
    AFT = mybir.ActivationFunctionType

    def _gat_patched(arch):
        t = {k: set(v) for k, v in _gat(arch).items()}
        if "sqrt_and_others" in t:
            for k in t:
                if k != "sqrt_and_others":
                    t[k].discard(AFT.Sqrt)
                    t[k].discard(AFT.Square)
        return t

    _bacc_mod.get_activation_tables = _gat_patched
    try:
        nc.finalize()
    finally:
        _bacc_mod.get_activation_tables = _gat
    return nc


def _host_inputs(inputs):
    X = np.asarray(inputs["X"], dtype=np.float32)
    W_Q = np.asarray(inputs["W_Q"], dtype=np.float32)
    b_Q = np.asarray(inputs["b_Q"], dtype=np.float32)
    W_K = np.asarray(inputs["W_K"], dtype=np.float32)
    b_K = np.asarray(inputs["b_K"], dtype=np.float32)
    W_V = np.asarray(inputs["W_V"], dtype=np.float32)
    b_V = np.asarray(inputs["b_V"], dtype=np.float32)
    sink = np.asarray(inputs["sink_scalars"], dtype=np.float32)
    v_nulls = np.asarray(inputs["v_nulls"], dtype=np.float32)
    W_O = np.asarray(inputs["W_O"], dtype=np.float32)

    XT = np.ascontiguousarray(X[0].T)

    # RoPE tables for interleaved channel-pair layout: row 2i -> (cos_i, -sin_i),
    # row 2i+1 -> (cos_i, +sin_i); partner = row ^ 1.
    invf = (1.0 / (10000.0 ** (np.arange(0, DH, 2, dtype=np.float32) / DH))
            ).astype(np.float32)
    freqs = np.arange(T, dtype=np.float32)[:, None] * invf[None, :]  # [T, 32]
    cos32 = np.cos(freqs).T  # [32, T]
    sin32 = np.sin(freqs).T
    cos64 = np.repeat(cos32, 2, axis=0)                      # [64, T]
    sin64 = np.stack([-sin32, sin32], axis=1).reshape(64, T)  # [64, T]
    cos128 = np.tile(cos64, (2, 1)).astype(np.float16)
    sin128 = np.tile(sin64, (2, 1)).astype(np.float16)

    sel = np.zeros((128, 2), dtype=np.float16)
    sel[0:64, 0] = 1.0
    sel[64:128, 1] = 1.0
    selt = np.zeros((2, 128), dtype=np.float16)
    selt[0, 0:64] = 1.0
    selt[1, 64:128] = 1.0

    in_maps = []
    for c in range(N_CORES):
        kcols = np.arange(128 * c, 128 * c + 128)
        wq_blocks, bq_cols = [], []
        for n in range(N_BR):
            qb = np.arange(1024 * n + 128 * c, 1024 * n + 128 * c + 128)
            wq_blocks.append(W_Q[:, qb])
            bq_cols.append(b_Q[qb])
        wq_full = np.concatenate(wq_blocks, axis=1)          # [1024, 512]
        bq_full = np.stack(bq_cols, axis=1)                  # [128, 4]
        # SBUF-image: [p, g, kt, m]
        wq_im = wq_full.reshape(KT, 128, 4, 128).transpose(1, 2, 0, 3)
        wq_im = wq_im.reshape(128, 4 * KT * 128)
        wo_full = np.concatenate(
            [0.25 * W_O[n, 128 * c:128 * c + 128, :] for n in range(N_BR)],
            axis=0,
        )                                                    # [512, 1024]
        # SBUF-image: [p, ct, mt, m]
        wo_im = wo_full.reshape(4, 128, 8, 128).transpose(1, 0, 2, 3)
        wo_im = wo_im.reshape(128, 4 * 8 * 128)
        heads = np.array([16 * n + 2 * c + u for n in range(N_BR)
                          for u in range(2)])
        sinks = sink[heads]
        tb = (S * (sinks + 1e-6)).astype(np.float32)[None, :]
        vns = np.zeros((64, 8), dtype=np.float32)
        for n in range(N_BR):
            for u in range(2):
                hl = 2 * n + u
                vns[:, hl] = S * sinks[hl] * v_nulls[n].reshape(N_HEAD, DH)[
                    2 * c + u
                ]
        in_maps.append(
            {
                "XT": XT.astype(np.float16),
                "WQ": np.ascontiguousarray(wq_im).astype(np.float16),
                "BQ": np.ascontiguousarray(bq_full).astype(np.float32),
                "WK": np.ascontiguousarray(
                    W_K[:, kcols].reshape(KT, 128, 128).transpose(1, 0, 2)
                    .reshape(128, KT * 128)).astype(np.float16),
                "BK": np.ascontiguousarray(b_K[kcols][:, None]).astype(
                    np.float32
                ),
                "WV": np.ascontiguousarray(
                    W_V[:, kcols].reshape(KT, 128, 128).transpose(1, 0, 2)
                    .reshape(128, KT * 128)).astype(np.float16),
                "BV": np.ascontiguousarray(b_V[kcols])[None, :].astype(
                    np.float16
                ),
                "WO": np.ascontiguousarray(wo_im).astype(np.float16),
                "COS": cos128,
                "SIN": sin128,
                "SEL": sel,
                "SELT": selt,
                "TB32": tb,
                "VNS": vns,
                "ONES": np.ones((1, T), dtype=np.float16),
                "TRI": np.tril(np.ones((128, 128))).astype(np.float16).T.copy(),
            }
        )
    return in_maps


def kernel(**inputs) -> np.ndarray:
    from concourse.bass_utils import run_bass_kernel_spmd

    in_maps = _host_inputs(inputs)
    if _NC_CACHE[0] is None:
        _NC_CACHE[0] = _build_nc()
    nc = _NC_CACHE[0]
    trace = bool(os.environ.get("KBENCH_TRACE"))
    res = run_bass_kernel_spmd(
        nc, in_maps, core_ids=list(range(N_CORES)), trace=trace
    )
    LAST_RESULT[0] = res
    if trace and res.exec_time_ns is not None:
        print(f"HW exec time: {res.exec_time_ns} ns")

    W_O_bias = np.asarray(inputs["W_O_bias"], dtype=np.float32)
    y = np.zeros((T, D_MODEL), dtype=np.float32)
    for r in res.results:
        y += np.asarray(r["YT"], dtype=np.float32).T
    y += W_O_bias.mean(axis=0)[None, :]
    return y[None, :, :]


# revision 8
# speedup vs baseline: 1.0515x; 1.0075x over previous
"""Trainium2 Bass kernel v2 for nn_Attention_65609920414302.

Same math as the baseline (see kernel.py docstring) with a restructured
schedule aimed at keeping the PE continuously busy (pstate ramp) and cutting
DVE/ACT/gpsimd totals:

- RoPE partner swap via DVE stream_shuffle (channel pairs interleaved on
  partitions, partner = p^1) instead of PE swap-matmuls + PSW tile.
- Per-key score scale m = sqrt(C2)/sqrt(DH*key_self) folded into krope
  (one broadcast + one multiply) so the Square activation runs with a
  scalar scale/bias and batches across key-block pairs: 5 ACT ops per
  (wave, head) instead of 8, packed into [128,1024] PSUM tiles.
- wbuf column order groups key blocks (0),(1,7),(2,6),(3,5),(4) so paired
  blocks share one PSUM tile and one Square op.
- +delta applied as ONE tensor_scalar per (wave, head) over [128,4608].
- sink offset tb rides the PV accumulation as a 1-partition matmul
  (lhsT=tb, rhs=ones) instead of a DVE add; 1/total read straight from
  PSUM with reciprocal_approx_fast.
- W_O merged into one end pass (no y_acc staging), fp16 weights/ctx,
  fp16 output casts on gpsimd, fp16 YT writeback.
- Scores of wave j interleave with PV of wave j-1 and with V/Q2/Q3
  projection fillers in PE issue order.
"""

import math
import os
import numpy as np

D_MODEL = 1024
N_HEAD = 16
N_BR = 4
DH = 64
T = 1024
S = math.pi / math.sqrt(3.0)
# minimax quadratic fit of g(x) = silu(S*softplus(x)) over x in [-0.70, 0.70]
C2 = 0.30301553
C1 = 0.90500395
C0 = 0.97984591
SQ_BETA = C1 / (2.0 * math.sqrt(C2))
SQ_DELTA = C0 - SQ_BETA * SQ_BETA
N_CORES = 8
KT = 8

# wbuf column layout: block order (0),(1,7),(2,6),(3,5),(4)
# tiles: list of (blocks, widths-within-tile)
SC_TILES = [
    [(0, 0)],            # block, col-offset inside tile
    [(1, 0), (7, 896)],
    [(2, 0), (6, 768)],
    [(3, 0), (5, 640)],
    [(4, 0)],
]
TILE_W = [1024, 1024, 1024, 1024, 512]
TILE_OFF = [0, 1024, 2048, 3072, 4096]
BOFF = {}
for _ti, _blks in enumerate(SC_TILES):
    for _b, _o in _blks:
        BOFF[_b] = TILE_OFF[_ti] + _o
W_COLS = 4608

XMASK = [i ^ 1 for i in range(32)]

_NC_CACHE = [None]
LAST_RESULT = [None]


def _build_nc():
    import concourse.bass as bass
    from concourse import bacc
    import concourse.mybir as mybir
    import concourse.tile as tile

    F32 = mybir.dt.float32
    F16 = mybir.dt.float16
    AF = mybir.ActivationFunctionType
    ALU = mybir.AluOpType

    nc = bacc.Bacc(None, target_bir_lowering=False, debug=False)

    XT = nc.declare_dram_parameter("XT", [D_MODEL, T], F16, isOutput=False)
    WQ = nc.declare_dram_parameter("WQ", [128, 4 * KT * 128], F16, isOutput=False)
    BQ = nc.declare_dram_parameter("BQ", [128, 4], F32, isOutput=False)
    WK = nc.declare_dram_parameter("WK", [128, KT * 128], F16, isOutput=False)
    BK = nc.declare_dram_parameter("BK", [128, 1], F32, isOutput=False)
    WV = nc.declare_dram_parameter("WV", [128, KT * 128], F16, isOutput=False)
    BV = nc.declare_dram_parameter("BV", [1, 128], F16, isOutput=False)
    WO = nc.declare_dram_parameter("WO", [128, 4 * 8 * 128], F16, isOutput=False)
    COS = nc.declare_dram_parameter("COS", [128, T], F16, isOutput=False)
    SIN = nc.declare_dram_parameter("SIN", [128, T], F16, isOutput=False)
    SEL = nc.declare_dram_parameter("SEL", [128, 2], F16, isOutput=False)
    SELT = nc.declare_dram_parameter("SELT", [2, 128], F16, isOutput=False)
    TB32 = nc.declare_dram_parameter("TB32", [1, 8], F32, isOutput=False)
    VNS = nc.declare_dram_parameter("VNS", [64, 8], F32, isOutput=False)
    ONES = nc.declare_dram_parameter("ONES", [1, T], F16, isOutput=False)
    TRI = nc.declare_dram_parameter("TRI", [128, 128], F16, isOutput=False)
    YT = nc.declare_dram_parameter("YT", [D_MODEL, T], F16, isOutput=True)

    with tile.TileContext(nc) as tc:
        pc = tc.alloc_tile_pool(name="const", bufs=1)
        pk = tc.alloc_tile_pool(name="keep", bufs=1)
        tr = tc.alloc_tile_pool(name="trans", bufs=2)
        pw = tc.alloc_tile_pool(name="wbuf", bufs=1)
        pa = tc.alloc_tile_pool(name="ps", bufs=1, space="PSUM")

        # ---- constants / weights in SBUF ----
        cos_sb = pc.tile([128, T], F16)
        sin_sb = pc.tile([128, T], F16)
        sel_sb = pc.tile([128, 2], F16)
        selt_sb = pc.tile([2, 128], F16)
        tb_sb = pc.tile([1, 8], F32)
        vns_sb = pc.tile([64, 8], F32)
        ones_r = pc.tile([1, T], F16)
        beta_sb = pc.tile([128, 1], F32)
        tri_sb = pc.tile([128, 128], F16)
        nc.vector.memset(beta_sb, SQ_BETA)
        warm = pc.tile([1, 1], F32)
        nc.vector.memset(warm, 1.0)
        nc.scalar.activation(warm, warm, AF.Sqrt)
        # PE warm-up: ~20 dummy matmuls on memset constants (no DMA deps)
        # keep the PE busy from t~2us so the frequency governor is already
        # ramped when the DMA-gated projections start at t~11us
        warm64 = pc.tile([1, 512], F16)
        warmw = pc.tile([1, 1], F16)
        nc.vector.memset(warm64, 1.0)
        nc.vector.memset(warmw, 1.0)
        warm_ps = pa.tile([128, T], F32, tag="sc", bufs=2, name="warm_ps")
        for _ in range(14):
            nc.tensor.matmul(warm_ps[0:1, 0:512], warmw, warm64,
                             start=True, stop=True)
        warm_rd = pc.tile([1, 64], F32)
        nc.vector.tensor_copy(warm_rd, warm_ps[0:1, 0:64])

        xt = pk.tile([128, KT, T], F16)
        wk = pk.tile([128, KT, 128], F16)
        bk = pk.tile([128, 1], F32)
        wq = pk.tile([128, 4, KT, 128], F16)
        bq = pk.tile([128, 4], F32)
        wv = pk.tile([128, KT, 128], F16)
        bv = pk.tile([1, 128], F16)
        wo = pk.tile([128, 4, 8, 128], F16)

        # DMA order: K-proj deps first, then Q/rope, V, W_O last.
        xt_src = XT.ap().rearrange("(kt p) t -> p kt t", p=128)
        wk_src = WK.ap().rearrange("(kt p) m -> p kt m", p=128)
        wq_src = WQ.ap().rearrange("(kt p) (g m) -> p kt g m", p=128, m=128)
        wv_src = WV.ap().rearrange("(kt p) v -> p kt v", p=128)
        # DMA-in: single issue stream, strict priority order. All queues
        # share bandwidth; descriptors drain roughly in issue order, so the
        # K-projection critical path (wk, xt) must be issued first.
        wq_im = WQ.ap().rearrange("p (g kt m) -> p g kt m", g=4, m=128)
        wo_im = WO.ap().rearrange("p (ct mt m) -> p ct mt m", ct=4, m=128)
        nc.sync.dma_start(
            out=wk, in_=WK.ap().rearrange("p (kt m) -> p kt m", m=128))
        nc.sync.dma_start(out=bk, in_=BK.ap())
        for kt in range(KT):
            nc.sync.dma_start(out=xt[:, kt, :], in_=xt_src[:, kt, :])
        nc.sync.dma_start(out=wq[:, 0, :, :], in_=wq_im[:, 0, :, :])
        nc.sync.dma_start(out=cos_sb, in_=COS.ap())
        nc.sync.dma_start(out=sin_sb, in_=SIN.ap())
        nc.sync.dma_start(out=sel_sb, in_=SEL.ap())
        nc.sync.dma_start(out=selt_sb, in_=SELT.ap())
        nc.sync.dma_start(out=bq, in_=BQ.ap())
        nc.sync.dma_start(out=wq[:, 1, :, :], in_=wq_im[:, 1, :, :])
        nc.sync.dma_start(
            out=wv, in_=WV.ap().rearrange("p (kt m) -> p kt m", m=128))
        nc.sync.dma_start(out=bv, in_=BV.ap())
        nc.sync.dma_start(out=ones_r, in_=ONES.ap())
        nc.sync.dma_start(out=wq[:, 2, :, :], in_=wq_im[:, 2, :, :])
        nc.sync.dma_start(out=wq[:, 3, :, :], in_=wq_im[:, 3, :, :])
        nc.sync.dma_start(out=tri_sb, in_=TRI.ap())
        nc.sync.dma_start(out=tb_sb, in_=TB32.ap())
        nc.sync.dma_start(out=vns_sb, in_=VNS.ap())
        nc.sync.dma_start(out=wo, in_=wo_im)

        krope = pk.tile([128, T], F16)
        qrope = pk.tile([128, 4, T], F16)
        m16 = pk.tile([2, T], F16)
        vstore = pk.tile([128, 8, 2, 65], F16)
        ctx = pk.tile([128, 4, T], F16)
        nc.vector.memset(vstore[:, :, :, 64:65], 1.0)

        # ---- projection emitters (PE part / vector part split) ----
        def proj_mm(w_ap):
            ps = pa.tile([128, T], F32, tag="sc", bufs=2)
            for th in range(2):
                sl = slice(512 * th, 512 * (th + 1))
                for kt in range(KT):
                    nc.tensor.matmul(
                        ps[:, sl], w_ap(kt), xt[:, kt, sl],
                        start=(kt == 0), stop=(kt == KT - 1),
                    )
            return ps

        def rope_vec(ps, b_col, out_ap, add_engine, k2_out=None):
            qsb = tr.tile([128, T], F16, tag="qsb", bufs=2)
            nc.vector.tensor_scalar_add(qsb, ps, b_col)
            if k2_out is not None:
                nc.vector.tensor_tensor(k2_out, qsb, qsb, op=ALU.mult)
            sw = tr.tile([128, T], F16, tag="sw", bufs=2)
            nc.vector.stream_shuffle(sw, qsb, XMASK)
            t1 = tr.tile([128, T], F16, tag="t1", bufs=2)
            nc.vector.tensor_tensor(t1, qsb, cos_sb, op=ALU.mult)
            t2 = tr.tile([128, T], F16, tag="t2", bufs=2)
            nc.vector.tensor_tensor(t2, sw, sin_sb, op=ALU.mult)
            add_engine.tensor_tensor(out_ap, t1, t2, op=ALU.add)

        # ---- startup: K proj, Q0/Q1 proj, key_self -> m -> krope scaled ----
        ps_k = proj_mm(lambda kt: wk[:, kt, :])
        k2 = tr.tile([128, T], F16, tag="k2", bufs=1)
        rope_vec(ps_k, bk[:, 0:1], krope, nc.vector, k2_out=k2)

        ks_ps = pa.tile([2, T], F32, tag="pv", bufs=2)
        for th in range(2):
            sl = slice(512 * th, 512 * (th + 1))
            nc.tensor.matmul(ks_ps[:, sl], sel_sb, k2[:, sl],
                             start=True, stop=True)
        ps_q0 = proj_mm(lambda kt: wq[:, 0, kt, :])
        m32 = tr.tile([2, T], F32, tag="m32", bufs=1)
        nc.vector.reciprocal_approx_fast(m32, ks_ps)
        # m = sqrt(C2/DH * 1/key_self)
        nc.scalar.activation(m16, m32, AF.Sqrt, scale=C2 / DH)

        rope_vec(ps_q0, bq[:, 0:1], qrope[:, 0, :], nc.vector)
        ps_q1 = proj_mm(lambda kt: wq[:, 1, kt, :])
        # broadcast m16 rows to 64-partition halves with one PE matmul
        mb_ps = pa.tile([128, T], F32, tag="sc", bufs=2, name="mb_ps")
        for th in range(2):
            sl = slice(512 * th, 512 * (th + 1))
            nc.tensor.matmul(mb_ps[:, sl], selt_sb, m16[:, sl],
                             start=True, stop=True)
        # krope scaled in place
        nc.vector.tensor_tensor(krope, krope, mb_ps, op=ALU.mult)
        rope_vec(ps_q1, bq[:, 1:2], qrope[:, 1, :], nc.vector)

        # ---- wave machinery ----
        wbuf_of = {}

        def emit_scores_tile(j, u, ti):
            """PE matmuls for one PSUM score tile + its Square."""
            wbuf = wbuf_of[j]
            r0 = 64 * u
            st = pa.tile([128, T], F32, tag="sc", bufs=2)
            w = TILE_W[ti]
            for (b, off) in SC_TILES[ti]:
                t0 = 128 * b
                L = T - t0
                c0 = 0
                while c0 < L:
                    # stay within 512-col PSUM bank regions of the tile
                    c1 = min(c0 + 512 - ((off + c0) % 512), L)
                    nc.tensor.matmul(
                        st[:, off + c0:off + c1],
                        krope[r0:r0 + 64, t0:t0 + 128],
                        qrope[r0:r0 + 64, j, t0 + c0:t0 + c1],
                        start=True, stop=True,
                    )
                    c0 = c1
            nc.scalar.activation(
                wbuf[:, u, TILE_OFF[ti]:TILE_OFF[ti] + w], st[:, 0:w],
                AF.Square, scale=1.0, bias=beta_sb[:, 0:1],
            )

        def emit_post_scores(j, u):
            """delta add (DVE) + causal masks split across DVE/gpsimd."""
            wbuf = wbuf_of[j]
            nc.vector.tensor_scalar_add(wbuf[:, u, :], wbuf[:, u, :], SQ_DELTA)
            for b in range(8):
                o = BOFF[b]
                nc.vector.tensor_tensor(
                    wbuf[:, u, o:o + 128], wbuf[:, u, o:o + 128], tri_sb,
                    op=ALU.mult,
                )

        pv_ps = {}

        def emit_pv_units(j, u):
            """PE matmul units for PV of (wave j, head u): [regionA, regionB]."""
            wbuf = wbuf_of[j]
            h = 2 * j + u
            ps_pv = pa.tile([65, T], F32, tag="pv", bufs=2)
            pv_ps[(j, u)] = ps_pv

            def regionA():
                first = True
                for b in (0, 1, 2):
                    t0 = 128 * b
                    nc.tensor.matmul(
                        ps_pv[:, t0:512],
                        vstore[:, b, u, :],
                        wbuf[:, u, BOFF[b]:BOFF[b] + (512 - t0)],
                        start=first, stop=False,
                    )
                    first = False
                nc.tensor.matmul(
                    ps_pv[:, 384:512],
                    vstore[:, 3, u, :],
                    wbuf[:, u, BOFF[3]:BOFF[3] + 128],
                    start=False, stop=True,
                )

            def regionB():
                first = True
                for b in range(8):
                    t0 = 128 * b
                    lo = max(512, t0)
                    nc.tensor.matmul(
                        ps_pv[:, lo:T],
                        vstore[:, b, u, :],
                        wbuf[:, u, BOFF[b] + (lo - t0):BOFF[b] + (T - t0)],
                        start=first, stop=(b == 7),
                    )
                    first = False

            return [regionA, regionB]

        def emit_pv_post(j, u):
            """recip (DVE) -> gb (gpsimd) -> ctx stt (DVE)."""
            h = 2 * j + u
            r0 = 64 * u
            ps_pv = pv_ps[(j, u)]
            tt = tr.tile([1, T], F32, tag="tt", bufs=2)
            nc.scalar.activation(tt, ps_pv[64:65, :], AF.Identity,
                                 bias=tb_sb[0:1, h:h + 1])
            tp = tr.tile([1, T], F32, tag="tp", bufs=2)
            nc.vector.reciprocal_approx_fast(tp, tt)
            gb = tr.tile([64, T], F32, tag="gb", bufs=2)
            nc.gpsimd.partition_broadcast(gb, tp, channels=64)
            nc.vector.scalar_tensor_tensor(
                out=ctx[r0:r0 + 64, j, :], in0=ps_pv[0:64, :],
                scalar=vns_sb[:, h:h + 1], in1=gb,
                op0=ALU.add, op1=ALU.mult,
            )

        # ---- V projection units (PE) + copies (DVE) ----
        def v_unit(tt_i):
            def emit():
                psv = pa.tile([128, T], F32, tag="sc", bufs=2)
                for kt in range(KT):
                    nc.tensor.matmul(
                        psv[:, 0:128], xt[:, kt, 128 * tt_i:128 * (tt_i + 1)],
                        wv[:, kt, :], start=(kt == 0), stop=False,
                    )
                nc.tensor.matmul(
                    psv[:, 0:128], ones_r[0:1, 0:128], bv,
                    start=False, stop=True,
                )
                nc.vector.tensor_copy(
                    vstore[:, tt_i, :, 0:64],
                    psv[:, 0:128].rearrange("p (h d) -> p h d", d=64),
                )
            return emit

        # ---- Q2/Q3 projection split into PE quarter-units + vector part ----
        def q_quarter(g, th, kts):
            def emit():
                ps = qps[g]
                sl = slice(512 * th, 512 * (th + 1))
                for kt in kts:
                    nc.tensor.matmul(
                        ps[:, sl], wq[:, g, kt, :], xt[:, kt, sl],
                        start=(kt == 0), stop=(kt == KT - 1),
                    )
            return emit

        qps = {}

        # ---------------- wave 0 ----------------
        wbuf_of[0] = pw.tile([128, 2, W_COLS], F16, tag="wbuf", bufs=3,
                             name="wbuf0")
        qps[2] = pa.tile([128, T], F32, tag="pv", bufs=2, name="qps2")
        fillers = [q_quarter(2, th, kts) for th in range(2)
                   for kts in (range(0, 4), range(4, 8))]
        fillers += [v_unit(i) for i in range(8)]
        fi = 0
        for u in range(2):
            for pos in range(5):
                ti = (4, 0, 1, 2, 3)[pos]
                if u == 1 and pos == 0 and fi < len(fillers):
                    fillers[fi]()
                    fi += 1
                emit_scores_tile(0, u, ti)
                if fi < len(fillers):
                    fillers[fi]()
                    fi += 1
                if (u == 1 or pos == 4) and fi < len(fillers):
                    fillers[fi]()
                    fi += 1
            emit_post_scores(0, u)
            if u == 0:
                rope_vec(qps[2], bq[:, 2:3], qrope[:, 2, :], nc.gpsimd)
        while fi < len(fillers):
            fillers[fi]()
            fi += 1

        # ---------------- waves 1..3 ----------------
        for j in (1, 2, 3):
            wbuf_of[j] = pw.tile([128, 2, W_COLS], F16, tag="wbuf", bufs=3,
                                 name=f"wbuf{j}")
            fillers = []
            if j == 1:
                qps[3] = pa.tile([128, T], F32, tag="pv", bufs=2, name="qps3")
                fillers = [q_quarter(3, th, kts) for th in range(2)
                           for kts in (range(0, 4), range(4, 8))]
            units0 = emit_pv_units(j - 1, 0)
            units1 = emit_pv_units(j - 1, 1)
            seq = fillers + [units0[0], units0[1], ("post", 0),
                             units1[0], units1[1], ("post", 1)]
            si = 0

            def take():
                nonlocal si, seq
                if si < len(seq):
                    it = seq[si]
                    si += 1
                    if isinstance(it, tuple):
                        # emit each post right after its units so the
                        # recip/gb/stt chain enters the DVE/gpsimd queues
                        # ahead of the next delta+masks and frees the PV
                        # PSUM slot a half-wave earlier
                        emit_pv_post(j - 1, it[1])
                    else:
                        it()

            for u in range(2):
                for pos in range(5):
                    ti = (4, 0, 1, 2, 3)[pos]
                    if u == 1 and pos == 0:
                        # run the PV unit BEFORE the half's first tile: it
                        # executes while ACT drains the u0 squares that this
                        # tile's PSUM slot is waiting on
                        take()
                    emit_scores_tile(j, u, ti)
                    if j == 1:
                        take()
                    elif (u == 0 and pos in (1, 3, 4)) or                          (u == 1 and pos in (2, 4)):
                        take()
                emit_post_scores(j, u)
                if j == 1 and u == 0:
                    rope_vec(qps[3], bq[:, 3:4], qrope[:, 3, :], nc.gpsimd)
            while si < len(seq):
                take()

        # ---------------- PV of wave 3 + W_O ----------------
        for unit in emit_pv_units(3, 0):
            unit()
        emit_pv_post(3, 0)
        # two W_O chains (branches 0-2 parts) started on the free sc slots:
        # they execute while wave-3 u1's mask/PV/normalize chain drains
        pre_ps = []
        for bi in range(2):
            mt, th = bi // 2, bi % 2
            sl = slice(512 * th, 512 * (th + 1))
            ps_o = pa.tile([128, 512], F32, tag="sc", bufs=2, name="ps_o")
            for ci in range(3):
                nc.tensor.matmul(
                    ps_o, wo[:, ci, mt, :], ctx[:, ci, sl],
                    start=(ci == 0), stop=False,
                )
            pre_ps.append((mt, sl, ps_o))
        for unit in emit_pv_units(3, 1):
            unit()
        # two more chains on the pv slots freed by stt(3,0) mid-drain
        for bi in range(2, 4):
            mt, th = bi // 2, bi % 2
            sl = slice(512 * th, 512 * (th + 1))
            ps_o = pa.tile([128, 512], F32, tag="pv", bufs=2, name="ps_o")
            for ci in range(3):
                nc.tensor.matmul(
                    ps_o, wo[:, ci, mt, :], ctx[:, ci, sl],
                    start=(ci == 0), stop=False,
                )
            pre_ps.append((mt, sl, ps_o))
        emit_pv_post(3, 1)

        def wo_finish(mt, sl, ps_o):
            nc.tensor.matmul(
                ps_o, wo[:, 3, mt, :], ctx[:, 3, sl],
                start=False, stop=True,
            )
            ysb = tr.tile([128, 512], F16, tag="ysb", bufs=3)
            nc.vector.tensor_copy(ysb, ps_o)
            nc.sync.dma_start(
                out=YT.ap()[128 * mt:128 * (mt + 1), sl], in_=ysb
            )

        for (mt, sl, ps_o) in pre_ps:
            wo_finish(mt, sl, ps_o)
        for bi in range(4, 16):
            mt, th = bi // 2, bi % 2
            sl = slice(512 * th, 512 * (th + 1))
            tag = "sc" if bi % 2 == 1 else "pv"
            ps_o = pa.tile([128, 512], F32, tag=tag, bufs=2, name="ps_o")
            for ci in range(4):
                nc.tensor.matmul(
                    ps_o, wo[:, ci, mt, :], ctx[:, ci, sl],
                    start=(ci == 0), stop=(ci == 3),
                )
            ysb = tr.tile([128, 512], F16, tag="ysb", bufs=3)
            nc.vector.tensor_copy(ysb, ps_o)
            nc.sync.dma_start(
                out=YT.ap()[128 * mt:128 * (mt + 1), sl], in_=ysb
            )

        pa.release()
        pw.release()
        tr.release()
        pk.release()
        pc.release()

    # pin Sqrt+Square to one table set so the picker never splits them
    import concourse.bacc as _bacc_mod
    from concourse.hw_specs import get_activation_tables as _gat
    AFT = mybir.ActivationFunctionType

    def _gat_patched(arch):
        t = {k: set(v) for k, v in _gat(arch).items()}
        if "sqrt_and_others" in t:
            for k in t:
                if k != "sqrt_and_others":
                    t[k].discard(AFT.Sqrt)
                    t[k].discard(AFT.Square)
        return t

    _bacc_mod.get_activation_tables = _gat_patched
    try:
        nc.finalize()
    finally:
        _bacc_mod.get_activation_tables = _gat
    return nc


def _host_inputs(inputs):
    X = np.asarray(inputs["X"], dtype=np.float32)
    W_Q = np.asarray(inputs["W_Q"], dtype=np.float32)
    b_Q = np.asarray(inputs["b_Q"], dtype=np.float32)
    W_K = np.asarray(inputs["W_K"], dtype=np.float32)
    b_K = np.asarray(inputs["b_K"], dtype=np.float32)
    W_V = np.asarray(inputs["W_V"], dtype=np.float32)
    b_V = np.asarray(inputs["b_V"], dtype=np.float32)
    sink = np.asarray(inputs["sink_scalars"], dtype=np.float32)
    v_nulls = np.asarray(inputs["v_nulls"], dtype=np.float32)
    W_O = np.asarray(inputs["W_O"], dtype=np.float32)

    XT = np.ascontiguousarray(X[0].T)

    # RoPE tables for interleaved channel-pair layout: row 2i -> (cos_i, -sin_i),
    # row 2i+1 -> (cos_i, +sin_i); partner = row ^ 1.
    invf = (1.0 / (10000.0 ** (np.arange(0, DH, 2, dtype=np.float32) / DH))
            ).astype(np.float32)
    freqs = np.arange(T, dtype=np.float32)[:, None] * invf[None, :]  # [T, 32]
    cos32 = np.cos(freqs).T  # [32, T]
    sin32 = np.sin(freqs).T
    cos64 = np.repeat(cos32, 2, axis=0)                      # [64, T]
    sin64 = np.stack([-sin32, sin32], axis=1).reshape(64, T)  # [64, T]
    cos128 = np.tile(cos64, (2, 1)).astype(np.float16)
    sin128 = np.tile(sin64, (2, 1)).astype(np.float16)

    sel = np.zeros((128, 2), dtype=np.float16)
    sel[0:64, 0] = 1.0
    sel[64:128, 1] = 1.0
    selt = np.zeros((2, 128), dtype=np.float16)
    selt[0, 0:64] = 1.0
    selt[1, 64:128] = 1.0

    in_maps = []
    for c in range(N_CORES):
        kcols = np.arange(128 * c, 128 * c + 128)
        wq_blocks, bq_cols = [], []
        for n in range(N_BR):
            qb = np.arange(1024 * n + 128 * c, 1024 * n + 128 * c + 128)
            wq_blocks.append(W_Q[:, qb])
            bq_cols.append(b_Q[qb])
        wq_full = np.concatenate(wq_blocks, axis=1)          # [1024, 512]
        bq_full = np.stack(bq_cols, axis=1)                  # [128, 4]
        # SBUF-image: [p, g, kt, m]
        wq_im = wq_full.reshape(KT, 128, 4, 128).transpose(1, 2, 0, 3)
        wq_im = wq_im.reshape(128, 4 * KT * 128)
        wo_full = np.concatenate(
            [0.25 * W_O[n, 128 * c:128 * c + 128, :] for n in range(N_BR)],
            axis=0,
        )                                                    # [512, 1024]
        # SBUF-image: [p, ct, mt, m]
        wo_im = wo_full.reshape(4, 128, 8, 128).transpose(1, 0, 2, 3)
        wo_im = wo_im.reshape(128, 4 * 8 * 128)
        heads = np.array([16 * n + 2 * c + u for n in range(N_BR)
                          for u in range(2)])
        sinks = sink[heads]
        tb = (S * (sinks + 1e-6)).astype(np.float32)[None, :]
        vns = np.zeros((64, 8), dtype=np.float32)
        for n in range(N_BR):
            for u in range(2):
                hl = 2 * n + u
                vns[:, hl] = S * sinks[hl] * v_nulls[n].reshape(N_HEAD, DH)[
                    2 * c + u
                ]
        in_maps.append(
            {
                "XT": XT.astype(np.float16),
                "WQ": np.ascontiguousarray(wq_im).astype(np.float16),
                "BQ": np.ascontiguousarray(bq_full).astype(np.float32),
                "WK": np.ascontiguousarray(
                    W_K[:, kcols].reshape(KT, 128, 128).transpose(1, 0, 2)
                    .reshape(128, KT * 128)).astype(np.float16),
                "BK": np.ascontiguousarray(b_K[kcols][:, None]).astype(
                    np.float32
                ),
                "WV": np.ascontiguousarray(
                    W_V[:, kcols].reshape(KT, 128, 128).transpose(1, 0, 2)
                    .reshape(128, KT * 128)).astype(np.float16),
                "BV": np.ascontiguousarray(b_V[kcols])[None, :].astype(
                    np.float16
                ),
                "WO": np.ascontiguousarray(wo_im).astype(np.float16),
                "COS": cos128,
                "SIN": sin128,
                "SEL": sel,
                "SELT": selt,
                "TB32": tb,
                "VNS": vns,
                "ONES": np.ones((1, T), dtype=np.float16),
                "TRI": np.tril(np.ones((128, 128))).astype(np.float16).T.copy(),
            }
        )
    return in_maps


def kernel(**inputs) -> np.ndarray:
    from concourse.bass_utils import run_bass_kernel_spmd

    in_maps = _host_inputs(inputs)
    if _NC_CACHE[0] is None:
        _NC_CACHE[0] = _build_nc()
    nc = _NC_CACHE[0]
    trace = bool(os.environ.get("KBENCH_TRACE"))
    res = run_bass_kernel_spmd(
        nc, in_maps, core_ids=list(range(N_CORES)), trace=trace
    )
    LAST_RESULT[0] = res
    if trace and res.exec_time_ns is not None:
        print(f"HW exec time: {res.exec_time_ns} ns")

    W_O_bias = np.asarray(inputs["W_O_bias"], dtype=np.float32)
    y = np.zeros((T, D_MODEL), dtype=np.float32)
    for r in res.results:
        y += np.asarray(r["YT"], dtype=np.float32).T
    y += W_O_bias.mean(axis=0)[None, :]
    return y[None, :, :]


# revision 9
# speedup vs baseline: 1.0585x; 1.0067x over previous
"""Trainium2 Bass kernel v2 for nn_Attention_65609920414302.

Same math as the baseline (see kernel.py docstring) with a restructured
schedule aimed at keeping the PE continuously busy (pstate ramp) and cutting
DVE/ACT/gpsimd totals:

- RoPE partner swap via DVE stream_shuffle (channel pairs interleaved on
  partitions, partner = p^1) instead of PE swap-matmuls + PSW tile.
- Per-key score scale m = sqrt(C2)/sqrt(DH*key_self) folded into krope
  (one broadcast + one multiply) so the Square activation runs with a
  scalar scale/bias and batches across key-block pairs: 5 ACT ops per
  (wave, head) instead of 8, packed into [128,1024] PSUM tiles.
- wbuf column order groups key blocks (0),(1,7),(2,6),(3,5),(4) so paired
  blocks share one PSUM tile and one Square op.
- +delta applied as ONE tensor_scalar per (wave, head) over [128,4608].
- sink offset tb rides the PV accumulation as a 1-partition matmul
  (lhsT=tb, rhs=ones) instead of a DVE add; 1/total read straight from
  PSUM with reciprocal_approx_fast.
- W_O merged into one end pass (no y_acc staging), fp16 weights/ctx,
  fp16 output casts on gpsimd, fp16 YT writeback.
- Scores of wave j interleave with PV of wave j-1 and with V/Q2/Q3
  projection fillers in PE issue order.
"""

import math
import os
import numpy as np

D_MODEL = 1024
N_HEAD = 16
N_BR = 4
DH = 64
T = 1024
S = math.pi / math.sqrt(3.0)
# minimax quadratic fit of g(x) = silu(S*softplus(x)) over x in [-0.70, 0.70]
C2 = 0.30301553
C1 = 0.90500395
C0 = 0.97984591
SQ_BETA = C1 / (2.0 * math.sqrt(C2))
SQ_DELTA = C0 - SQ_BETA * SQ_BETA
N_CORES = 8
KT = 8

# wbuf column layout: block order (0),(1,7),(2,6),(3,5),(4)
# tiles: list of (blocks, widths-within-tile)
SC_TILES = [
    [(0, 0)],            # block, col-offset inside tile
    [(1, 0), (7, 896)],
    [(2, 0), (6, 768)],
    [(3, 0), (5, 640)],
    [(4, 0)],
]
TILE_W = [1024, 1024, 1024, 1024, 512]
TILE_OFF = [0, 1024, 2048, 3072, 4096]
BOFF = {}
for _ti, _blks in enumerate(SC_TILES):
    for _b, _o in _blks:
        BOFF[_b] = TILE_OFF[_ti] + _o
W_COLS = 4608

XMASK = [i ^ 1 for i in range(32)]

_NC_CACHE = [None]
LAST_RESULT = [None]


def _build_nc():
    import concourse.bass as bass
    from concourse import bacc
    import concourse.mybir as mybir
    import concourse.tile as tile

    F32 = mybir.dt.float32
    F16 = mybir.dt.float16
    AF = mybir.ActivationFunctionType
    ALU = mybir.AluOpType

    nc = bacc.Bacc(None, target_bir_lowering=False, debug=False)

    XT = nc.declare_dram_parameter("XT", [D_MODEL, T], F16, isOutput=False)
    WQ = nc.declare_dram_parameter("WQ", [128, 4 * KT * 128], F16, isOutput=False)
    BQ = nc.declare_dram_parameter("BQ", [128, 4], F32, isOutput=False)
    WK = nc.declare_dram_parameter("WK", [128, KT * 128], F16, isOutput=False)
    BK = nc.declare_dram_parameter("BK", [128, 1], F32, isOutput=False)
    WV = nc.declare_dram_parameter("WV", [128, KT * 128], F16, isOutput=False)
    BV = nc.declare_dram_parameter("BV", [1, 128], F16, isOutput=False)
    WO = nc.declare_dram_parameter("WO", [128, 4 * 8 * 128], F16, isOutput=False)
    COS = nc.declare_dram_parameter("COS", [128, T], F16, isOutput=False)
    SIN = nc.declare_dram_parameter("SIN", [128, T], F16, isOutput=False)
    SEL = nc.declare_dram_parameter("SEL", [128, 2], F16, isOutput=False)
    SELT = nc.declare_dram_parameter("SELT", [2, 128], F16, isOutput=False)
    TB32 = nc.declare_dram_parameter("TB32", [1, 8], F32, isOutput=False)
    VNS = nc.declare_dram_parameter("VNS", [64, 8], F32, isOutput=False)
    ONES = nc.declare_dram_parameter("ONES", [1, T], F16, isOutput=False)
    TRI = nc.declare_dram_parameter("TRI", [128, 128], F16, isOutput=False)
    YT = nc.declare_dram_parameter("YT", [D_MODEL, T], F16, isOutput=True)

    with tile.TileContext(nc) as tc:
        pc = tc.alloc_tile_pool(name="const", bufs=1)
        pk = tc.alloc_tile_pool(name="keep", bufs=1)
        tr = tc.alloc_tile_pool(name="trans", bufs=2)
        pw = tc.alloc_tile_pool(name="wbuf", bufs=1)
        pa = tc.alloc_tile_pool(name="ps", bufs=1, space="PSUM")

        # ---- constants / weights in SBUF ----
        cos_sb = pc.tile([128, T], F16)
        sin_sb = pc.tile([128, T], F16)
        sel_sb = pc.tile([128, 2], F16)
        selt_sb = pc.tile([2, 128], F16)
        tb_sb = pc.tile([1, 8], F32)
        vns_sb = pc.tile([64, 8], F32)
        ones_r = pc.tile([1, T], F16)
        beta_sb = pc.tile([128, 1], F32)
        tri_sb = pc.tile([128, 128], F16)
        nc.vector.memset(beta_sb, SQ_BETA)
        warm = pc.tile([1, 1], F32)
        nc.vector.memset(warm, 1.0)
        nc.scalar.activation(warm, warm, AF.Sqrt)
        # PE warm-up: ~20 dummy matmuls on memset constants (no DMA deps)
        # keep the PE busy from t~2us so the frequency governor is already
        # ramped when the DMA-gated projections start at t~11us
        warm64 = pc.tile([1, 512], F16)
        warmw = pc.tile([1, 1], F16)
        nc.vector.memset(warm64, 1.0)
        nc.vector.memset(warmw, 1.0)
        warm_ps = pa.tile([128, T], F32, tag="sc", bufs=2, name="warm_ps")
        for _ in range(14):
            nc.tensor.matmul(warm_ps[0:1, 0:512], warmw, warm64,
                             start=True, stop=True)
        warm_rd = pc.tile([1, 64], F32)
        nc.vector.tensor_copy(warm_rd, warm_ps[0:1, 0:64])

        xt = pk.tile([128, KT, T], F16)
        wk = pk.tile([128, KT, 128], F16)
        bk = pk.tile([128, 1], F32)
        wq = pk.tile([128, 4, KT, 128], F16)
        bq = pk.tile([128, 4], F32)
        wv = pk.tile([128, KT, 128], F16)
        bv = pk.tile([1, 128], F16)
        wo = pk.tile([128, 4, 8, 128], F16)

        # DMA order: K-proj deps first, then Q/rope, V, W_O last.
        xt_src = XT.ap().rearrange("(kt p) t -> p kt t", p=128)
        wk_src = WK.ap().rearrange("(kt p) m -> p kt m", p=128)
        wq_src = WQ.ap().rearrange("(kt p) (g m) -> p kt g m", p=128, m=128)
        wv_src = WV.ap().rearrange("(kt p) v -> p kt v", p=128)
        # DMA-in: single issue stream, strict priority order. All queues
        # share bandwidth; descriptors drain roughly in issue order, so the
        # K-projection critical path (wk, xt) must be issued first.
        wq_im = WQ.ap().rearrange("p (g kt m) -> p g kt m", g=4, m=128)
        wo_im = WO.ap().rearrange("p (ct mt m) -> p ct mt m", ct=4, m=128)
        nc.sync.dma_start(
            out=wk, in_=WK.ap().rearrange("p (kt m) -> p kt m", m=128))
        nc.sync.dma_start(out=bk, in_=BK.ap())
        for kt in range(KT):
            nc.sync.dma_start(out=xt[:, kt, :], in_=xt_src[:, kt, :])
        nc.sync.dma_start(out=wq[:, 0, :, :], in_=wq_im[:, 0, :, :])
        nc.sync.dma_start(out=cos_sb, in_=COS.ap())
        nc.sync.dma_start(out=sin_sb, in_=SIN.ap())
        nc.sync.dma_start(out=sel_sb, in_=SEL.ap())
        nc.sync.dma_start(out=selt_sb, in_=SELT.ap())
        nc.sync.dma_start(out=bq, in_=BQ.ap())
        nc.sync.dma_start(out=wq[:, 1, :, :], in_=wq_im[:, 1, :, :])
        nc.sync.dma_start(
            out=wv, in_=WV.ap().rearrange("p (kt m) -> p kt m", m=128))
        nc.sync.dma_start(out=bv, in_=BV.ap())
        nc.sync.dma_start(out=ones_r, in_=ONES.ap())
        nc.sync.dma_start(out=wq[:, 2, :, :], in_=wq_im[:, 2, :, :])
        nc.sync.dma_start(out=wq[:, 3, :, :], in_=wq_im[:, 3, :, :])
        nc.sync.dma_start(out=tri_sb, in_=TRI.ap())
        nc.sync.dma_start(out=tb_sb, in_=TB32.ap())
        nc.sync.dma_start(out=vns_sb, in_=VNS.ap())
        nc.sync.dma_start(out=wo, in_=wo_im)

        krope = pk.tile([128, T], F16)
        qrope = pk.tile([128, 4, T], F16)
        m16 = pk.tile([2, T], F16)
        vstore = pk.tile([128, 8, 2, 65], F16)
        ctx = pk.tile([128, 4, T], F16)
        nc.vector.memset(vstore[:, :, :, 64:65], 1.0)

        # ---- projection emitters (PE part / vector part split) ----
        def proj_mm(w_ap):
            ps = pa.tile([128, T], F32, tag="sc", bufs=2)
            for th in range(2):
                sl = slice(512 * th, 512 * (th + 1))
                for kt in range(KT):
                    nc.tensor.matmul(
                        ps[:, sl], w_ap(kt), xt[:, kt, sl],
                        start=(kt == 0), stop=(kt == KT - 1),
                    )
            return ps

        def rope_vec(ps, b_col, out_ap, add_engine, k2_out=None):
            qsb = tr.tile([128, T], F16, tag="qsb", bufs=2)
            nc.vector.tensor_scalar_add(qsb, ps, b_col)
            if k2_out is not None:
                nc.vector.tensor_tensor(k2_out, qsb, qsb, op=ALU.mult)
            sw = tr.tile([128, T], F16, tag="sw", bufs=2)
            nc.vector.stream_shuffle(sw, qsb, XMASK)
            t1 = tr.tile([128, T], F16, tag="t1", bufs=2)
            nc.vector.tensor_tensor(t1, qsb, cos_sb, op=ALU.mult)
            t2 = tr.tile([128, T], F16, tag="t2", bufs=2)
            nc.vector.tensor_tensor(t2, sw, sin_sb, op=ALU.mult)
            add_engine.tensor_tensor(out_ap, t1, t2, op=ALU.add)

        # ---- startup: K proj, Q0/Q1 proj, key_self -> m -> krope scaled ----
        ps_k = proj_mm(lambda kt: wk[:, kt, :])
        k2 = tr.tile([128, T], F16, tag="k2", bufs=1)
        rope_vec(ps_k, bk[:, 0:1], krope, nc.vector, k2_out=k2)

        ks_ps = pa.tile([2, T], F32, tag="pv", bufs=2)
        for th in range(2):
            sl = slice(512 * th, 512 * (th + 1))
            nc.tensor.matmul(ks_ps[:, sl], sel_sb, k2[:, sl],
                             start=True, stop=True)
        ps_q0 = proj_mm(lambda kt: wq[:, 0, kt, :])
        m32 = tr.tile([2, T], F32, tag="m32", bufs=1)
        nc.vector.reciprocal_approx_fast(m32, ks_ps)
        # m = sqrt(C2/DH * 1/key_self)
        nc.scalar.activation(m16, m32, AF.Sqrt, scale=C2 / DH)

        rope_vec(ps_q0, bq[:, 0:1], qrope[:, 0, :], nc.vector)
        ps_q1 = proj_mm(lambda kt: wq[:, 1, kt, :])
        # broadcast m16 rows to 64-partition halves with one PE matmul
        mb_ps = pa.tile([128, T], F32, tag="sc", bufs=2, name="mb_ps")
        for th in range(2):
            sl = slice(512 * th, 512 * (th + 1))
            nc.tensor.matmul(mb_ps[:, sl], selt_sb, m16[:, sl],
                             start=True, stop=True)
        # krope scaled in place
        nc.vector.tensor_tensor(krope, krope, mb_ps, op=ALU.mult)
        rope_vec(ps_q1, bq[:, 1:2], qrope[:, 1, :], nc.vector)

        # ---- wave machinery ----
        wbuf_of = {}

        def emit_scores_tile(j, u, ti):
            """PE matmuls for one PSUM score tile + its Square."""
            wbuf = wbuf_of[j]
            r0 = 64 * u
            st = pa.tile([128, T], F32, tag="sc", bufs=2)
            w = TILE_W[ti]
            for (b, off) in SC_TILES[ti]:
                t0 = 128 * b
                L = T - t0
                c0 = 0
                while c0 < L:
                    # stay within 512-col PSUM bank regions of the tile
                    c1 = min(c0 + 512 - ((off + c0) % 512), L)
                    nc.tensor.matmul(
                        st[:, off + c0:off + c1],
                        krope[r0:r0 + 64, t0:t0 + 128],
                        qrope[r0:r0 + 64, j, t0 + c0:t0 + c1],
                        start=True, stop=True,
                    )
                    c0 = c1
            nc.scalar.activation(
                wbuf[:, u, TILE_OFF[ti]:TILE_OFF[ti] + w], st[:, 0:w],
                AF.Square, scale=1.0, bias=beta_sb[:, 0:1],
            )

        def emit_post_scores(j, u):
            """delta add (DVE) + causal masks split across DVE/gpsimd."""
            wbuf = wbuf_of[j]
            nc.vector.tensor_scalar_add(wbuf[:, u, :], wbuf[:, u, :], SQ_DELTA)
            for b in range(8):
                o = BOFF[b]
                nc.vector.tensor_tensor(
                    wbuf[:, u, o:o + 128], wbuf[:, u, o:o + 128], tri_sb,
                    op=ALU.mult,
                )

        pv_ps = {}

        def emit_pv_units(j, u):
            """PE matmul units for PV of (wave j, head u): [regionA, regionB]."""
            wbuf = wbuf_of[j]
            h = 2 * j + u
            ps_pv = pa.tile([65, T], F32, tag="pv", bufs=2)
            pv_ps[(j, u)] = ps_pv

            def regionA():
                first = True
                for b in (0, 1, 2):
                    t0 = 128 * b
                    nc.tensor.matmul(
                        ps_pv[:, t0:512],
                        vstore[:, b, u, :],
                        wbuf[:, u, BOFF[b]:BOFF[b] + (512 - t0)],
                        start=first, stop=False,
                    )
                    first = False
                nc.tensor.matmul(
                    ps_pv[:, 384:512],
                    vstore[:, 3, u, :],
                    wbuf[:, u, BOFF[3]:BOFF[3] + 128],
                    start=False, stop=True,
                )

            def regionB():
                first = True
                for b in range(8):
                    t0 = 128 * b
                    lo = max(512, t0)
                    nc.tensor.matmul(
                        ps_pv[:, lo:T],
                        vstore[:, b, u, :],
                        wbuf[:, u, BOFF[b] + (lo - t0):BOFF[b] + (T - t0)],
                        start=first, stop=(b == 7),
                    )
                    first = False

            return [regionA, regionB]

        def emit_pv_post(j, u):
            """recip (DVE) -> gb (gpsimd) -> ctx stt (DVE)."""
            h = 2 * j + u
            r0 = 64 * u
            ps_pv = pv_ps[(j, u)]
            tt = tr.tile([1, T], F32, tag="tt", bufs=2)
            nc.scalar.activation(tt, ps_pv[64:65, :], AF.Identity,
                                 bias=tb_sb[0:1, h:h + 1])
            tp = tr.tile([1, T], F32, tag="tp", bufs=2)
            nc.vector.reciprocal_approx_fast(tp, tt)
            gb = tr.tile([64, T], F32, tag="gb", bufs=2)
            nc.gpsimd.partition_broadcast(gb, tp, channels=64)
            nc.vector.scalar_tensor_tensor(
                out=ctx[r0:r0 + 64, j, :], in0=ps_pv[0:64, :],
                scalar=vns_sb[:, h:h + 1], in1=gb,
                op0=ALU.add, op1=ALU.mult,
            )

        # ---- V projection units (PE) + copies (DVE) ----
        def v_unit(tt_i):
            def emit():
                psv = pa.tile([128, T], F32, tag="sc", bufs=2)
                for kt in range(KT):
                    nc.tensor.matmul(
                        psv[:, 0:128], xt[:, kt, 128 * tt_i:128 * (tt_i + 1)],
                        wv[:, kt, :], start=(kt == 0), stop=False,
                    )
                nc.tensor.matmul(
                    psv[:, 0:128], ones_r[0:1, 0:128], bv,
                    start=False, stop=True,
                )
                nc.vector.tensor_copy(
                    vstore[:, tt_i, :, 0:64],
                    psv[:, 0:128].rearrange("p (h d) -> p h d", d=64),
                )
            return emit

        # ---- Q2/Q3 projection split into PE quarter-units + vector part ----
        def q_quarter(g, th, kts):
            def emit():
                ps = qps[g]
                sl = slice(512 * th, 512 * (th + 1))
                for kt in kts:
                    nc.tensor.matmul(
                        ps[:, sl], wq[:, g, kt, :], xt[:, kt, sl],
                        start=(kt == 0), stop=(kt == KT - 1),
                    )
            return emit

        qps = {}

        # ---------------- wave 0 ----------------
        wbuf_of[0] = pw.tile([128, 2, W_COLS], F16, tag="wbuf", bufs=3,
                             name="wbuf0")
        qps[2] = pa.tile([128, T], F32, tag="pv", bufs=2, name="qps2")
        fillers = [q_quarter(2, th, kts) for th in range(2)
                   for kts in (range(0, 4), range(4, 8))]
        fillers += [v_unit(i) for i in range(8)]
        fi = 0
        for u in range(2):
            for pos in range(5):
                ti = (4, 0, 1, 2, 3)[pos]
                if u == 1 and pos == 0 and fi < len(fillers):
                    fillers[fi]()
                    fi += 1
                emit_scores_tile(0, u, ti)
                if fi < len(fillers):
                    fillers[fi]()
                    fi += 1
                if (u == 1 or pos == 4) and fi < len(fillers):
                    fillers[fi]()
                    fi += 1
            emit_post_scores(0, u)
            if u == 0:
                rope_vec(qps[2], bq[:, 2:3], qrope[:, 2, :], nc.gpsimd)
        while fi < len(fillers):
            fillers[fi]()
            fi += 1

        # ---------------- waves 1..3 ----------------
        for j in (1, 2, 3):
            wbuf_of[j] = pw.tile([128, 2, W_COLS], F16, tag="wbuf", bufs=3,
                                 name=f"wbuf{j}")
            fillers = []
            if j == 1:
                qps[3] = pa.tile([128, T], F32, tag="pv", bufs=2, name="qps3")
                fillers = [q_quarter(3, th, kts) for th in range(2)
                           for kts in (range(0, 4), range(4, 8))]
            units0 = emit_pv_units(j - 1, 0)
            units1 = emit_pv_units(j - 1, 1)
            if j == 1:
                seq = fillers + [units0[0], units0[1], ("post", 0),
                                 units1[0], units1[1], ("post", 1)]
            else:
                seq = [units0[0], units0[1], ("post", 0),
                       units1[1], units1[0], ("post", 1)]
            si = 0

            def take():
                nonlocal si, seq
                if si < len(seq):
                    it = seq[si]
                    si += 1
                    if isinstance(it, tuple):
                        # emit each post right after its units so the
                        # recip/gb/stt chain enters the DVE/gpsimd queues
                        # ahead of the next delta+masks and frees the PV
                        # PSUM slot a half-wave earlier
                        emit_pv_post(j - 1, it[1])
                    else:
                        it()

            for u in range(2):
                for pos in range(5):
                    ti = (4, 0, 1, 2, 3)[pos]
                    if u == 1 and pos == 0:
                        # run the PV unit BEFORE the half's first tile: it
                        # executes while ACT drains the u0 squares that this
                        # tile's PSUM slot is waiting on
                        take()
                    emit_scores_tile(j, u, ti)
                    if j == 1:
                        take()
                    elif (u == 0 and pos in (1, 3, 4)) or                          (u == 1 and pos in (2, 4)):
                        take()
                emit_post_scores(j, u)
                if j == 1 and u == 0:
                    rope_vec(qps[3], bq[:, 3:4], qrope[:, 3, :], nc.gpsimd)
            while si < len(seq):
                take()

        # ---------------- PV of wave 3 + W_O ----------------
        for unit in emit_pv_units(3, 0):
            unit()
        emit_pv_post(3, 0)
        # two W_O chains (branches 0-2 parts) started on the free sc slots:
        # they execute while wave-3 u1's mask/PV/normalize chain drains
        pre_ps = []
        for bi in range(2):
            mt, th = bi // 2, bi % 2
            sl = slice(512 * th, 512 * (th + 1))
            ps_o = pa.tile([128, 512], F32, tag="sc", bufs=2, name="ps_o")
            for ci in range(3):
                nc.tensor.matmul(
                    ps_o, wo[:, ci, mt, :], ctx[:, ci, sl],
                    start=(ci == 0), stop=False,
                )
            pre_ps.append((mt, sl, ps_o))
        for unit in emit_pv_units(3, 1):
            unit()
        # two more chains on the pv slots freed by stt(3,0) mid-drain
        for bi in range(2, 4):
            mt, th = bi // 2, bi % 2
            sl = slice(512 * th, 512 * (th + 1))
            ps_o = pa.tile([128, 512], F32, tag="pv", bufs=2, name="ps_o")
            for ci in range(3):
                nc.tensor.matmul(
                    ps_o, wo[:, ci, mt, :], ctx[:, ci, sl],
                    start=(ci == 0), stop=False,
                )
            pre_ps.append((mt, sl, ps_o))
        emit_pv_post(3, 1)

        def wo_finish(mt, sl, ps_o):
            nc.tensor.matmul(
                ps_o, wo[:, 3, mt, :], ctx[:, 3, sl],
                start=False, stop=True,
            )
            ysb = tr.tile([128, 512], F16, tag="ysb", bufs=3)
            nc.vector.tensor_copy(ysb, ps_o)
            nc.sync.dma_start(
                out=YT.ap()[128 * mt:128 * (mt + 1), sl], in_=ysb
            )

        for (mt, sl, ps_o) in pre_ps:
            wo_finish(mt, sl, ps_o)
        for bi in range(4, 16):
            mt, th = bi // 2, bi % 2
            sl = slice(512 * th, 512 * (th + 1))
            tag = "sc" if bi % 2 == 1 else "pv"
            ps_o = pa.tile([128, 512], F32, tag=tag, bufs=2, name="ps_o")
            for ci in range(4):
                nc.tensor.matmul(
                    ps_o, wo[:, ci, mt, :], ctx[:, ci, sl],
                    start=(ci == 0), stop=(ci == 3),
                )
            ysb = tr.tile([128, 512], F16, tag="ysb", bufs=3)
            nc.vector.tensor_copy(ysb, ps_o)
            nc.sync.dma_start(
                out=YT.ap()[128 * mt:128 * (mt + 1), sl], in_=ysb
            )

        pa.release()
        pw.release()
        tr.release()
        pk.release()
        pc.release()

    # pin Sqrt+Square to one table set so the picker never splits them
    import concourse.bacc as _bacc_mod
    from concourse.hw_specs import get_activation_tables as _gat
    AFT = mybir.ActivationFunctionType

    def _gat_patched(arch):
        t = {k: set(v) for k, v in _gat(arch).items()}
        if "sqrt_and_others" in t:
            for k in t:
                if k != "sqrt_and_others":
                    t[k].discard(AFT.Sqrt)
                    t[k].discard(AFT.Square)
        return t

    _bacc_mod.get_activation_tables = _gat_patched
    try:
        nc.finalize()
    finally:
        _bacc_mod.get_activation_tables = _gat
    return nc


def _host_inputs(inputs):
    X = np.asarray(inputs["X"], dtype=np.float32)
    W_Q = np.asarray(inputs["W_Q"], dtype=np.float32)
    b_Q = np.asarray(inputs["b_Q"], dtype=np.float32)
    W_K = np.asarray(inputs["W_K"], dtype=np.float32)
    b_K = np.asarray(inputs["b_K"], dtype=np.float32)
    W_V = np.asarray(inputs["W_V"], dtype=np.float32)
    b_V = np.asarray(inputs["b_V"], dtype=np.float32)
    sink = np.asarray(inputs["sink_scalars"], dtype=np.float32)
    v_nulls = np.asarray(inputs["v_nulls"], dtype=np.float32)
    W_O = np.asarray(inputs["W_O"], dtype=np.float32)

    XT = np.ascontiguousarray(X[0].T)

    # RoPE tables for interleaved channel-pair layout: row 2i -> (cos_i, -sin_i),
    # row 2i+1 -> (cos_i, +sin_i); partner = row ^ 1.
    invf = (1.0 / (10000.0 ** (np.arange(0, DH, 2, dtype=np.float32) / DH))
            ).astype(np.float32)
    freqs = np.arange(T, dtype=np.float32)[:, None] * invf[None, :]  # [T, 32]
    cos32 = np.cos(freqs).T  # [32, T]
    sin32 = np.sin(freqs).T
    cos64 = np.repeat(cos32, 2, axis=0)                      # [64, T]
    sin64 = np.stack([-sin32, sin32], axis=1).reshape(64, T)  # [64, T]
    cos128 = np.tile(cos64, (2, 1)).astype(np.float16)
    sin128 = np.tile(sin64, (2, 1)).astype(np.float16)

    sel = np.zeros((128, 2), dtype=np.float16)
    sel[0:64, 0] = 1.0
    sel[64:128, 1] = 1.0
    selt = np.zeros((2, 128), dtype=np.float16)
    selt[0, 0:64] = 1.0
    selt[1, 64:128] = 1.0

    in_maps = []
    for c in range(N_CORES):
        kcols = np.arange(128 * c, 128 * c + 128)
        wq_blocks, bq_cols = [], []
        for n in range(N_BR):
            qb = np.arange(1024 * n + 128 * c, 1024 * n + 128 * c + 128)
            wq_blocks.append(W_Q[:, qb])
            bq_cols.append(b_Q[qb])
        wq_full = np.concatenate(wq_blocks, axis=1)          # [1024, 512]
        bq_full = np.stack(bq_cols, axis=1)                  # [128, 4]
        # SBUF-image: [p, g, kt, m]
        wq_im = wq_full.reshape(KT, 128, 4, 128).transpose(1, 2, 0, 3)
        wq_im = wq_im.reshape(128, 4 * KT * 128)
        wo_full = np.concatenate(
            [0.25 * W_O[n, 128 * c:128 * c + 128, :] for n in range(N_BR)],
            axis=0,
        )                                                    # [512, 1024]
        # SBUF-image: [p, ct, mt, m]
        wo_im = wo_full.reshape(4, 128, 8, 128).transpose(1, 0, 2, 3)
        wo_im = wo_im.reshape(128, 4 * 8 * 128)
        heads = np.array([16 * n + 2 * c + u for n in range(N_BR)
                          for u in range(2)])
        sinks = sink[heads]
        tb = (S * (sinks + 1e-6)).astype(np.float32)[None, :]
        vns = np.zeros((64, 8), dtype=np.float32)
        for n in range(N_BR):
            for u in range(2):
                hl = 2 * n + u
                vns[:, hl] = S * sinks[hl] * v_nulls[n].reshape(N_HEAD, DH)[
                    2 * c + u
                ]
        in_maps.append(
            {
                "XT": XT.astype(np.float16),
                "WQ": np.ascontiguousarray(wq_im).astype(np.float16),
                "BQ": np.ascontiguousarray(bq_full).astype(np.float32),
                "WK": np.ascontiguousarray(
                    W_K[:, kcols].reshape(KT, 128, 128).transpose(1, 0, 2)
                    .reshape(128, KT * 128)).astype(np.float16),
                "BK": np.ascontiguousarray(b_K[kcols][:, None]).astype(
                    np.float32
                ),
                "WV": np.ascontiguousarray(
                    W_V[:, kcols].reshape(KT, 128, 128).transpose(1, 0, 2)
                    .reshape(128, KT * 128)).astype(np.float16),
                "BV": np.ascontiguousarray(b_V[kcols])[None, :].astype(
                    np.float16
                ),
                "WO": np.ascontiguousarray(wo_im).astype(np.float16),
                "COS": cos128,
                "SIN": sin128,
                "SEL": sel,
                "SELT": selt,
                "TB32": tb,
                "VNS": vns,
                "ONES": np.ones((1, T), dtype=np.float16),
                "TRI": np.tril(np.ones((128, 128))).astype(np.float16).T.copy(),
            }
        )
    return in_maps


def kernel(**inputs) -> np.ndarray:
    from concourse.bass_utils import run_bass_kernel_spmd

    in_maps = _host_inputs(inputs)
    if _NC_CACHE[0] is None:
        _NC_CACHE[0] = _build_nc()
    nc = _NC_CACHE[0]
    trace = bool(os.environ.get("KBENCH_TRACE"))
    res = run_bass_kernel_spmd(
        nc, in_maps, core_ids=list(range(N_CORES)), trace=trace
    )
    LAST_RESULT[0] = res
    if trace and res.exec_time_ns is not None:
        print(f"HW exec time: {res.exec_time_ns} ns")

    W_O_bias = np.asarray(inputs["W_O_bias"], dtype=np.float32)
    y = np.zeros((T, D_MODEL), dtype=np.float32)
    for r in res.results:
        y += np.asarray(r["YT"], dtype=np.float32).T
    y += W_O_bias.mean(axis=0)[None, :]
    return y[None, :, :]
